# revision 61
# baseline (speedup 1.0000x reference)
"""Trainium2 Bass kernel for a dense transformer block (B=8, S=512, D=768, H=12, Fd=3072).

Sharding: pure data-parallel over batch - one batch element per NeuronCore,
weights replicated, no collectives.

Key structure (v2):
  - Attention projections (q,k,v), the attn@v contraction and the output
    projection run in fp8e4m3 with DoubleRow perf mode (2 k-tiles per pass,
    2x PE throughput).  Scores (K=64) and the FFN stay bf16: fp8 would
    blow the 2e-2 error budget (measured 2.2e-2 for fp8 FFN in simulation),
    while qkv+attn+proj fp8 sims at 1.15e-2 total.
  - scoresT[t,s] layout (keys on partitions) so the key-padding mask is a
    per-partition factor: it folds into v (v rows and the sum-column get
    multiplied by mask), not into the softmax input.
  - The Gaussian positional bias enters as a multiplicative factor
    exp(gauss[s,t]) on exp(scores).  Since gauss < 2.3e-6 beyond |t-s| >= 4,
    the factor is 1.0 outside a 264-wide diagonal band: one small in-place
    band multiply per score tile instead of a full [512,512] multiply.
  - exp is computed with a fixed -3.5 shift (ACT bias) so values fit fp8
    range (max ~80 < 240); the shift cancels in the softmax normalization.
  - softmax sums come from a mask-valued extra column appended to v; the
    per-column reciprocal is broadcast across partitions with a 1-row
    f32r matmul (f32r: 1 cycle/row at N=512, no cast needed).
  - v/proj biases fold into the residual input on the host:
    x + b_proj + bv @ w_proj.  bk drops entirely (a per-query score shift
    cancels in softmax).  bq rides the qT activation bias, b1 the gelu
    bias, b2 a K=1 ones-row matmul.
  - Big weight DMAs are triggered from the Pool (gpsimd) queue (25ns per
    trigger vs 667ns on ACT/DVE), ordered so the first qk matmul only
    waits for xT + the first th-slice of wq.
"""

import numpy as np
import ml_dtypes

import concourse.bass as bass
import concourse.mybir as mybir
import concourse.tile as tile
from concourse.tile import add_dep_helper
from concourse import bacc
from concourse import bass_utils
from concourse.masks import make_identity

BF = mybir.dt.bfloat16
F32 = mybir.dt.float32
F32R = mybir.dt.float32r
F8 = mybir.dt.float8e4
AF = mybir.ActivationFunctionType
OP = mybir.AluOpType
DR = mybir.MatmulPerfMode.DoubleRow

B, S, D, H, Dh, Fd = 8, 512, 768, 12, 64, 3072
NCORES = 8
EPS = 1e-12
SHIFT = 3.5        # exp(score - SHIFT): keeps fp8 exp values < 240
BAND = 264         # gauss factor band width (|t-s|<=4 significant)

KD = D // 128      # 6  K-tiles over D
KP = KD // 2       # 3  K-tile pairs (DoubleRow)
MS = S // 128      # 4  chunks over sequence
KF = Fd // 128     # 24 K-tiles over Fd
NT = 2             # N-tiles over D for natural-layout outputs (2 x 384)
ND = D // NT       # 384


def _trace(nc, io):
    with tile.TileContext(nc) as tc:
        _trace_body(nc, tc, io)


def _act_recip(nc, out, in_):
    """ACT-engine reciprocal (InstActivation, func=Reciprocal).

    The bass wrapper refuses Reciprocal on ACT for accuracy reasons; the
    softmax normalizer only needs ~1e-2 relative accuracy, which the ACT
    table comfortably provides (validated against the reference output).
    """
    eng = nc.scalar
    inputs = [eng.lower_ap(in_)]
    for val in (0.0, 1.0, 0.0):  # bias, scale, alpha immediates
        inputs.append(mybir.ImmediateValue(dtype=mybir.dt.float32, value=val))
    return eng.add_instruction(
        mybir.InstActivation(
            name=eng.bass.get_next_instruction_name(),
            func=AF.Reciprocal,
            ins=inputs,
            outs=[eng.lower_ap(out)],
        )
    )


def _trace_body(nc, tc, io):
    from contextlib import ExitStack

    with ExitStack() as ctx:
        glob = ctx.enter_context(tc.tile_pool(name="glob", bufs=1))

        # ---- constants / small inputs (scalar queue: tiny, needed early) ----
        ident_f = glob.tile([128, 128], F32, tag="ident")
        make_identity(nc, ident_f)
        ones_bf = glob.tile([1, 512], BF, tag="ones_bf")
        nc.vector.memset(ones_bf, 1.0)
        ones64_b = glob.tile([1, 64], BF, tag="ones64")
        nc.vector.memset(ones64_b, 1.0)
        eps_t = glob.tile([128, 1], F32, tag="eps")
        nc.vector.memset(eps_t, EPS)
        nshift_t = glob.tile([128, 1], F32, tag="nshift")
        nc.vector.memset(nshift_t, -SHIFT)
        zero_t = glob.tile([128, 1], F32, tag="zero")
        nc.vector.memset(zero_t, 0.0)

        biasf_sb = glob.tile([128, KD + KF], F32, tag="biasf")
        nc.scalar.dma_start(out=biasf_sb, in_=io["bias_f"])
        bq8_c = biasf_sb[:, 0:KD]
        b1_c = biasf_sb[:, KD : KD + KF]

        biasb_sb = glob.tile([1, D], BF, tag="biasb")
        nc.scalar.dma_start(out=biasb_sb, in_=io["bias_b"])
        b2_r = biasb_sb[:, 0:D]

        mrow_sb = glob.tile([128, MS], F32, tag="mrow")
        nc.scalar.dma_start(out=mrow_sb, in_=io["mrow"])

        gbt = glob.tile([128, 4, D], BF, tag="gbt")
        g1b, be1b, g2b, be2b = gbt[:, 0, :], gbt[:, 1, :], gbt[:, 2, :], gbt[:, 3, :]

        w1_sb = glob.tile([128, KD, Fd], BF, tag="w1")
        h1_sb = glob.tile([128, MS, D], F32, tag="h1")
        h1T_sb = glob.tile([128, KD, S], BF, tag="h1T")
        x_sb = glob.tile([128, MS, D], BF, tag="x")

        # ================= attention scope =================
        with tc.tile_pool(name="attn", bufs=1) as attnp:
            xT_sb = attnp.tile([128, KD, S], F8, tag="xT")
            # th-slice 0 as separate tiles: tile-granular dependency tracking
            # means the first qk matmuls would otherwise wait for the whole
            # wq/wk transfer.
            wq0_sb = attnp.tile([128, 1, KD, 128], F8, tag="wq0")
            wk0_sb = attnp.tile([128, 1, KD, 128], F8, tag="wk0")
            wqr_sb = attnp.tile([128, 5, KD, 128], F8, tag="wqr")
            wkr_sb = attnp.tile([128, 5, KD, 128], F8, tag="wkr")
            wv_sb = attnp.tile([128, KD, D], F8, tag="wv")
            wp_sb = attnp.tile([128, KD, D], F8, tag="wp")
            gband_sb = attnp.tile([128, 2, 2, BAND], BF, tag="gband")

            def wq_sl(th):
                return (wq0_sb if th == 0 else wqr_sb)[:, 0 if th == 0 else th - 1]

            def wk_sl(th):
                return (wk0_sb if th == 0 else wkr_sb)[:, 0 if th == 0 else th - 1]

            # big-lead DMAs: descriptor gen costs ~0.65us per trigger, so
            # order by first use: xT+wq0+wk0 (first matmuls), wv (v tiles),
            # gband (first exp), then the bulk qk weights.
            nc.gpsimd.dma_start(
                out=xT_sb.rearrange("p c s -> p (c s)"), in_=io["xT8"]
            )
            nc.gpsimd.dma_start(
                out=wq0_sb.rearrange("p t c n -> p (t c n)"), in_=io["wq8"][:, 0:D]
            )
            nc.gpsimd.dma_start(
                out=wk0_sb.rearrange("p t c n -> p (t c n)"), in_=io["wk8"][:, 0:D]
            )
            nc.gpsimd.dma_start(out=wv_sb.rearrange("p c n -> p (c n)"), in_=io["wv8"])
            nc.gpsimd.dma_start(
                out=gband_sb.rearrange("p a b w -> p (a b w)"), in_=io["gband"]
            )
            nc.gpsimd.dma_start(
                out=wqr_sb.rearrange("p t c n -> p (t c n)"),
                in_=io["wq8"][:, D : 6 * D],
            )
            nc.gpsimd.dma_start(
                out=wkr_sb.rearrange("p t c n -> p (t c n)"),
                in_=io["wk8"][:, D : 6 * D],
            )
            late_dmas = [
                nc.sync.dma_start(out=wp_sb.rearrange("p c n -> p (c n)"), in_=io["wp8"]),
                nc.sync.dma_start(out=gbt.rearrange("p c n -> p (c n)"), in_=io["gb"]),
                nc.sync.dma_start(out=x_sb.rearrange("p c n -> p (c n)"), in_=io["x"]),
            ]

            qT_sb = attnp.tile([128, KD, S], BF, tag="qT")
            kT_sb = attnp.tile([128, KD, S], BF, tag="kT")
            # 128-wide per-head slots: v | mask-col x64. The 64 replicated
            # mask columns make the attn@v matmul emit the softmax sums
            # broadcast across partitions 64..127 for free (PE cost depends
            # only on N), so normalization needs no separate broadcast.
            v_sb = attnp.tile([128, MS, H, 128], F8, tag="v")
            nc.gpsimd.memset(v_sb[:, :, :, Dh:128], 1.0)
            hT_sb = attnp.tile([128, KD, S], F8, tag="hT")

            with tc.tile_pool(name="psA", bufs=1, space="PSUM") as psA:

                def qk_tile(th):
                    # bk dropped: a per-query score shift cancels in softmax.
                    act = None
                    for w_sl, dst in ((wq_sl(th), qT_sb), (wk_sl(th), kT_sb)):
                        ps = psA.tile([128, 512], F32, tag="acc", bufs=2, name="ps_qk")
                        for j in range(KP):
                            nc.tensor.matmul(
                                ps, w_sl[:, 2 * j : 2 * j + 2, :],
                                xT_sb[:, 2 * j : 2 * j + 2, :],
                                start=(j == 0), stop=(j == KP - 1), perf_mode=DR,
                            )
                        if dst is qT_sb:
                            act = nc.scalar.activation(
                                out=dst[:, th, :], in_=ps, func=AF.Identity,
                                bias=bq8_c[:, th : th + 1], scale=0.125,
                            )
                        else:
                            act = nc.vector.tensor_copy(out=dst[:, th, :], in_=ps)
                    return act

                def v_tiles(n):
                    # bv host-folded into x; mask folded into v rows (and the
                    # sum column) so the softmax input needs no mask at all.
                    for c in range(MS):
                        ps = psA.tile([128, ND], F32, tag="acc", bufs=2, name="ps_v")
                        for j in range(KP):
                            nc.tensor.matmul(
                                ps, xT_sb[:, 2 * j : 2 * j + 2, 128 * c : 128 * (c + 1)],
                                wv_sb[:, 2 * j : 2 * j + 2, ND * n : ND * (n + 1)],
                                start=(j == 0), stop=(j == KP - 1), perf_mode=DR,
                            )
                        nc.vector.tensor_scalar(
                            out=v_sb[:, c, 6 * n : 6 * (n + 1), 0:Dh],
                            in0=ps.rearrange("p (h d) -> p h d", d=Dh),
                            scalar1=mrow_sb[:, c : c + 1], scalar2=None,
                            op0=OP.mult,
                        )
                    if n == 1:
                        for c in range(MS):
                            nc.vector.tensor_scalar(
                                out=v_sb[:, c, :, Dh:128],
                                in0=v_sb[:, c, :, Dh:128],
                                scalar1=mrow_sb[:, c : c + 1],
                                scalar2=None, op0=OP.mult,
                            )

                def scores_exp(h):
                    th, off = h // 2, (h % 2) * 64
                    qh = qT_sb[off : off + 64, th, :]
                    kh = kT_sb[off : off + 64, th, :]
                    exp_tiles = []
                    for half in range(2):
                        ps_sc = psA.tile([128, 2, 512], F32, tag="sc2", bufs=2, name="ps_sc")
                        for j in range(2):
                            c = 2 * half + j
                            nc.tensor.matmul(
                                ps_sc[:, j, :], kh[:, 128 * c : 128 * (c + 1)], qh,
                                start=True, stop=True,
                            )
                        ex = attnp.tile([128, 2, 512], F8, tag="exp", bufs=8, name="ex")
                        nc.scalar.activation(out=ex, in_=ps_sc, func=AF.Exp, bias=nshift_t)
                        # gauss factor: identity outside a 264-wide diagonal band
                        lo = 256 * half - 4
                        wlo, whi = max(0, -lo), min(BAND, 512 - lo)
                        eng = nc.gpsimd if half == 1 else nc.vector
                        eng.tensor_tensor(
                            out=ex[:, :, lo + wlo : lo + whi],
                            in0=ex[:, :, lo + wlo : lo + whi],
                            in1=gband_sb[:, half, :, wlo:whi], op=OP.mult,
                        )
                        exp_tiles.append(ex)
                    return exp_tiles

                def h_chain(h, exp_tiles):
                    ps_h = psA.tile([128, 512], F32, tag="hT", bufs=2, name="ps_h")
                    for half in range(2):
                        nc.tensor.matmul(
                            ps_h, v_sb[:, 2 * half : 2 * half + 2, h, :],
                            exp_tiles[half],
                            start=(half == 0), stop=(half == 1), perf_mode=DR,
                        )
                    # sums (rows 64..127, already partition-broadcast by the
                    # matmul) to SBUF, then a [64,512] reciprocal: approx
                    # cost is partition-blind, so this is as cheap as [1,512].
                    sums = attnp.tile([64, 512], F32, tag="sums", bufs=2, name="sums")
                    nc.scalar.copy(sums, ps_h[Dh:128, :])
                    rec = attnp.tile([64, 512], F32, tag="rec", bufs=2, name="rec")
                    nc.vector.reciprocal_approx_fast(out=rec, in_=sums)
                    return ps_h, rec

                def h_apply(h, ps_h, rec):
                    th, off = h // 2, (h % 2) * 64
                    return nc.vector.tensor_tensor(
                        out=hT_sb[off : off + 64, th, :], in0=ps_h[0:Dh, :],
                        in1=rec, op=OP.mult,
                    )

                # software-pipelined head loop: per iteration each engine
                # sees at most one stage of each kind, so stage latencies of
                # consecutive heads overlap instead of serializing.
                qk_acts = [qk_tile(0), qk_tile(1)]
                # not needed until proj/LN1: keep them off the DMA rings
                # while the attention lead-in streams in
                for dma in late_dmas:
                    add_dep_helper(dma.ins, qk_acts[0].ins, True, "defer until lead-in clear")
                exps, chains = {}, {}
                exps[0] = scores_exp(0)
                v_tiles(0)
                exps[1] = scores_exp(1)
                v_tiles(1)
                last_mult = None
                for h in range(H + 2):
                    if h < H and h not in exps:
                        exps[h] = scores_exp(h)
                        if 2 <= h <= 5:
                            qk_acts.append(qk_tile(h))
                    if 1 <= h <= H:
                        chains[h - 1] = h_chain(h - 1, exps.pop(h - 1))
                    if h >= 2:
                        last_mult = h_apply(h - 2, *chains.pop(h - 2))

                # defer the big FFN weight DMAs until the lead-in is clear
                w1dma = nc.sync.dma_start(out=w1_sb.rearrange("p c n -> p (c n)"), in_=io["w1_bf"])
                add_dep_helper(w1dma.ins, qk_acts[-1].ins, True, "defer w1 until qkT done")

                # --- proj (fp8 DoubleRow) + residual + LN1 + h1 transpose,
                # software-pipelined: transposes of chunk m-1 fill the PE
                # while chunk m's LN chain runs on DVE/ACT ---
                def proj_chunk(m):
                    pss = []
                    for n in range(NT):
                        ps = psA.tile([128, ND], F32, tag="acc", bufs=2, name="ps_pr")
                        for j in range(KP):
                            nc.tensor.matmul(
                                ps, hT_sb[:, 2 * j : 2 * j + 2, 128 * m : 128 * (m + 1)],
                                wp_sb[:, 2 * j : 2 * j + 2, ND * n : ND * (n + 1)],
                                start=(j == 0), stop=(j == KP - 1), perf_mode=DR,
                            )
                        pss.append(ps)
                    row = glob.tile([128, D], F32, tag="rowtmp", bufs=2, name="row")
                    for n in range(NT):
                        nc.vector.tensor_tensor(
                            out=row[:, ND * n : ND * (n + 1)], in0=pss[n],
                            in1=x_sb[:, m, ND * n : ND * (n + 1)], op=OP.add,
                        )
                    _layernorm(nc, glob, row, g1b, be1b, eps_t, h1_sb[:, m, :])

                def transpose_chunk(m):
                    for f in range(KD):
                        ps_t = psA.tile([128, 128], F32, tag="acc", bufs=2, name="ps_t")
                        nc.tensor.transpose(ps_t, h1_sb[:, m, 128 * f : 128 * (f + 1)], ident_f)
                        nc.scalar.copy(out=h1T_sb[:, f, 128 * m : 128 * (m + 1)], in_=ps_t)

                proj_chunk(0)
                proj_chunk(1)
                transpose_chunk(0)
                proj_chunk(2)
                transpose_chunk(1)
                proj_chunk(3)
                transpose_chunk(2)
                transpose_chunk(3)

        # ================= FFN scope (bf16) =================
        with tc.tile_pool(name="ffn", bufs=1) as ffnp, \
             tc.tile_pool(name="psF", bufs=1, space="PSUM") as psF:
            ff1T_sb = ffnp.tile([128, KF, S], BF, tag="ff1T")
            w2_sb = ffnp.tile([128, KF, D], BF, tag="w2")
            w2dma = nc.sync.dma_start(out=w2_sb.rearrange("p c n -> p (c n)"), in_=io["w2_bf"])
            add_dep_helper(w2dma.ins, last_mult.ins, True, "defer w2 until attention done")
            # two half-S passes: the first needs only h1 chunks 0-1, so it
            # overlaps the LN1/transpose ramp of chunks 2-3.
            for half in range(2):
                sl = slice(256 * half, 256 * (half + 1))
                for fm in range(KF):
                    ps = psF.tile([128, 256], F32, tag="acc", bufs=8, name="ps_f1")
                    for k in range(KD):
                        nc.tensor.matmul(
                            ps, w1_sb[:, k, 128 * fm : 128 * (fm + 1)],
                            h1T_sb[:, k, sl],
                            start=(k == 0), stop=(k == KD - 1),
                        )
                    nc.scalar.activation(
                        out=ff1T_sb[:, fm, sl], in_=ps, func=AF.Gelu,
                        bias=b1_c[:, fm : fm + 1], scale=1.0,
                    )

            for m in range(MS):
                pss = []
                for n in range(NT):
                    ps = psF.tile([128, ND], F32, tag="acc", bufs=8, name="ps_f2")
                    for k in range(KF):
                        nc.tensor.matmul(
                            ps, ff1T_sb[:, k, 128 * m : 128 * (m + 1)],
                            w2_sb[:, k, ND * n : ND * (n + 1)],
                            start=(k == 0), stop=False,
                        )
                    nc.tensor.matmul(
                        ps, ones_bf[:, 0:128], b2_r[:, ND * n : ND * (n + 1)],
                        start=False, stop=True,
                    )
                    pss.append(ps)
                row = glob.tile([128, D], F32, tag="rowtmp", bufs=2, name="row2")
                for n in range(NT):
                    nc.vector.tensor_tensor(
                        out=row[:, ND * n : ND * (n + 1)], in0=pss[n],
                        in1=h1_sb[:, m, ND * n : ND * (n + 1)], op=OP.add,
                    )
                outrow = glob.tile([128, D], F32, tag="outrow", bufs=2, name="outrow")
                _layernorm(nc, glob, row, g2b, be2b, eps_t, outrow)
                nc.gpsimd.dma_start(
                    out=io["out"][128 * m : 128 * (m + 1), :], in_=outrow
                )


def _layernorm(nc, pool, row, gamma_b, beta_b, eps_t, out_ap):
    st = pool.tile([128, 3, 6], F32, tag="st", bufs=2, name="st")
    for g in range(3):
        nc.vector.bn_stats(out=st[:, g, :], in_=row[:, 256 * g : 256 * (g + 1)])
    mv = pool.tile([128, 2], F32, tag="mv", bufs=2, name="mv")
    nc.vector.bn_aggr(out=mv, in_=st)
    sd = pool.tile([128, 1], F32, tag="sd", bufs=2, name="sd")
    nc.scalar.activation(out=sd, in_=mv[:, 1:2], func=AF.Sqrt, bias=eps_t, scale=1.0)
    rs = pool.tile([128, 1], F32, tag="rs", bufs=2, name="rs")
    nc.vector.reciprocal(rs, sd)
    # in-place: row = (row - mean) * gamma ; out = row * rstd + beta
    nc.vector.scalar_tensor_tensor(
        out=row, in0=row, scalar=mv[:, 0:1], in1=gamma_b, op0=OP.subtract, op1=OP.mult
    )
    nc.vector.scalar_tensor_tensor(
        out=out_ap, in0=row, scalar=rs, in1=beta_b, op0=OP.mult, op1=OP.add
    )


_SPECS = [
    # (name, shape, dtype) - big tensors pre-permuted on host to SBUF layout
    ("x", [128, MS * D], BF),
    ("xT8", [128, KD * S], F8),
    ("wq8", [128, 6 * D], F8),       # th-major: [p, th, k, 128]
    ("wk8", [128, 6 * D], F8),
    ("wv8", [128, KD * D], F8),
    ("wp8", [128, KD * D], F8),
    ("gband", [128, 2 * 2 * BAND], BF),
    ("mrow", [128, MS], F32),
    ("w1_bf", [128, KD * Fd], BF),
    ("w2_bf", [128, KF * D], BF),
    ("bias_f", [128, KD + KF], F32),   # bq8 | b1, per-partition cols
    ("bias_b", [1, D], BF),            # b2 row
    ("gb", [128, 4 * D], BF),          # gamma1|beta1|gamma2|beta2 (host-bcast)
]

_BUILT = {}


def _build():
    if "nc" in _BUILT:
        return _BUILT["nc"]
    nc = bacc.Bacc("TRN2", target_bir_lowering=False, debug=False,
                   enable_asserts=False, num_devices=NCORES)
    io = {}
    for name, shape, dt in _SPECS:
        io[name] = nc.dram_tensor(name, shape, dt, kind="ExternalInput").ap()
    io["out"] = nc.dram_tensor("out", [S, D], F32, kind="ExternalOutput").ap()
    _trace(nc, io)
    nc.compile()
    _BUILT["nc"] = nc
    return nc


def _host_prep(inputs):
    bf = ml_dtypes.bfloat16
    f8 = ml_dtypes.float8_e4m3
    f32 = np.float32
    x = np.asarray(inputs["x"], f32)
    mask = np.asarray(inputs["mask"])

    def q8(a):
        return np.asarray(np.clip(a, -240.0, 240.0), f8)

    idx = np.arange(S, dtype=np.float64)
    dd = idx[None, :] - idx[:, None]
    sc = -0.5 * dd * dd
    sc -= sc.max(axis=-1, keepdims=True)
    e = np.exp(sc)
    gauss = e / e.sum(axis=-1, keepdims=True)  # [query s, key t], float64

    # band factor table: gband[t_loc, half, j, w] = exp(gauss[s, t]),
    # with t = 256*half + 128*j + t_loc, s = 256*half - 4 + w (1.0 if s OOB)
    t_loc = np.arange(128)
    gband = np.ones((128, 2, 2, BAND), np.float64)
    for half in range(2):
        for j in range(2):
            t_abs = 256 * half + 128 * j + t_loc          # [128]
            s_abs = 256 * half - 4 + np.arange(BAND)      # [BAND]
            valid = (s_abs >= 0) & (s_abs < S)
            gband[:, half, j, valid] = np.exp(gauss[s_abs[valid][None, :], t_abs[:, None]])
    gband = np.ascontiguousarray(gband.reshape(128, -1)).astype(bf)

    def sbl(a, p=128):  # [C*p, N] -> [p, C*N] (SBUF layout)
        cN = a.shape[0] // p
        return np.ascontiguousarray(
            a.reshape(cN, p, a.shape[1]).transpose(1, 0, 2).reshape(p, -1)
        )

    def thmaj(w):  # [D, D] -> [128, th, k, 128] flat (th-major fp8)
        a = np.asarray(w, f32).reshape(KD, 128, KD, 128)
        return np.ascontiguousarray(
            q8(a.transpose(1, 2, 0, 3)).reshape(128, -1)
        )

    def pcols(a, p=128):  # [C*p] -> [p, C] per-partition columns
        return np.ascontiguousarray(a.reshape(-1, p).T)

    bias_f = np.concatenate(
        [
            pcols(np.asarray(inputs["bq"], f32) * np.float32(0.125)),
            pcols(np.asarray(inputs["b1"], f32)),
        ],
        axis=1,
    )
    bias_b = np.asarray(inputs["b2"], f32).astype(bf)[None, :]
    # v/proj biases fold into the residual: x + h@wproj + bproj + bv@wproj
    x_fold = (
        np.asarray(inputs["b_proj"], f32)
        + np.asarray(inputs["bv"], f32) @ np.asarray(inputs["w_proj"], f32)
    )
    gb = np.ascontiguousarray(
        np.broadcast_to(
            np.concatenate(
                [
                    np.asarray(inputs["gamma1"], f32),
                    np.asarray(inputs["beta1"], f32),
                    np.asarray(inputs["gamma2"], f32),
                    np.asarray(inputs["beta2"], f32),
                ]
            )[None, :],
            (128, 4 * D),
        )
    ).astype(bf)
    shared = {
        "wq8": thmaj(inputs["wq"]),
        "wk8": thmaj(inputs["wk"]),
        "wv8": sbl(q8(np.asarray(inputs["wv"], f32))),
        "wp8": sbl(q8(np.asarray(inputs["w_proj"], f32))),
        "w1_bf": sbl(np.asarray(inputs["w1"], f32).astype(bf)),
        "w2_bf": sbl(np.asarray(inputs["w2"], f32).astype(bf)),
        "gband": gband,
        "bias_f": bias_f,
        "bias_b": bias_b,
        "gb": gb,
    }
    in_maps = []
    for b in range(NCORES):
        m = dict(shared)
        m["x"] = sbl(np.ascontiguousarray(x[b] + x_fold[None, :]).astype(bf))
        m["xT8"] = sbl(q8(np.ascontiguousarray(x[b].T)))
        m["mrow"] = np.ascontiguousarray(
            mask[b].astype(f32).reshape(MS, 128).T
        )
        in_maps.append(m)
    return in_maps


def _run(inputs, trace=False, trace_cores=None):
    nc = _build()
    in_maps = _host_prep(inputs)
    res = bass_utils.run_bass_kernel_spmd(
        nc, in_maps, core_ids=list(range(NCORES)), trace=trace,
        trace_cores=trace_cores,
    )
    out = np.stack([np.asarray(res.results[b]["out"]) for b in range(NCORES)])
    return out.astype(np.float32), res


def kernel(**inputs):
    return _run(inputs)[0]


# revision 62
# speedup vs baseline: 1.1893x; 1.1893x over previous
"""Trainium2 Bass kernel for a dense transformer block (B=8, S=512, D=768, H=12, Fd=3072).

Sharding: pure data-parallel over batch - one batch element per NeuronCore,
weights replicated, no collectives.

Key structure (v2):
  - Attention projections (q,k,v), the attn@v contraction and the output
    projection run in fp8e4m3 with DoubleRow perf mode (2 k-tiles per pass,
    2x PE throughput).  Scores (K=64) and the FFN stay bf16: fp8 would
    blow the 2e-2 error budget (measured 2.2e-2 for fp8 FFN in simulation),
    while qkv+attn+proj fp8 sims at 1.15e-2 total.
  - scoresT[t,s] layout (keys on partitions) so the key-padding mask is a
    per-partition factor: it folds into v (v rows and the sum-column get
    multiplied by mask), not into the softmax input.
  - The Gaussian positional bias enters as a multiplicative factor
    exp(gauss[s,t]) on exp(scores).  Since gauss < 2.3e-6 beyond |t-s| >= 4,
    the factor is 1.0 outside a 264-wide diagonal band: one small in-place
    band multiply per score tile instead of a full [512,512] multiply.
  - exp is computed with a fixed -3.5 shift (ACT bias) so values fit fp8
    range (max ~80 < 240); the shift cancels in the softmax normalization.
  - softmax sums come from a mask-valued extra column appended to v; the
    per-column reciprocal is broadcast across partitions with a 1-row
    f32r matmul (f32r: 1 cycle/row at N=512, no cast needed).
  - v/proj biases fold into the residual input on the host:
    x + b_proj + bv @ w_proj.  bk drops entirely (a per-query score shift
    cancels in softmax).  bq rides the qT activation bias, b1 the gelu
    bias, b2 a K=1 ones-row matmul.
  - Big weight DMAs are triggered from the Pool (gpsimd) queue (25ns per
    trigger vs 667ns on ACT/DVE), ordered so the first qk matmul only
    waits for xT + the first th-slice of wq.
"""

import numpy as np
import ml_dtypes

import concourse.bass as bass
import concourse.mybir as mybir
import concourse.tile as tile
from concourse.tile import add_dep_helper
from concourse import bacc
from concourse import bass_utils
from concourse.masks import make_identity

BF = mybir.dt.bfloat16
F32 = mybir.dt.float32
F32R = mybir.dt.float32r
F8 = mybir.dt.float8e4
AF = mybir.ActivationFunctionType
OP = mybir.AluOpType
DR = mybir.MatmulPerfMode.DoubleRow

B, S, D, H, Dh, Fd = 8, 512, 768, 12, 64, 3072
NCORES = 8
EPS = 1e-12
SHIFT = 3.5        # exp(score - SHIFT): keeps fp8 exp values < 240
BAND = 264         # gauss factor band width (|t-s|<=4 significant)

KD = D // 128      # 6  K-tiles over D
KP = KD // 2       # 3  K-tile pairs (DoubleRow)
MS = S // 128      # 4  chunks over sequence
KF = Fd // 128     # 24 K-tiles over Fd
NT = 2             # N-tiles over D for natural-layout outputs (2 x 384)
ND = D // NT       # 384


def _trace(nc, io):
    with tile.TileContext(nc) as tc:
        _trace_body(nc, tc, io)


def _act_recip(nc, out, in_):
    """ACT-engine reciprocal (InstActivation, func=Reciprocal).

    The bass wrapper refuses Reciprocal on ACT for accuracy reasons; the
    softmax normalizer only needs ~1e-2 relative accuracy, which the ACT
    table comfortably provides (validated against the reference output).
    """
    eng = nc.scalar
    inputs = [eng.lower_ap(in_)]
    for val in (0.0, 1.0, 0.0):  # bias, scale, alpha immediates
        inputs.append(mybir.ImmediateValue(dtype=mybir.dt.float32, value=val))
    return eng.add_instruction(
        mybir.InstActivation(
            name=eng.bass.get_next_instruction_name(),
            func=AF.Reciprocal,
            ins=inputs,
            outs=[eng.lower_ap(out)],
        )
    )


def _trace_body(nc, tc, io):
    from contextlib import ExitStack

    with ExitStack() as ctx:
        glob = ctx.enter_context(tc.tile_pool(name="glob", bufs=1))

        # ---- constants / small inputs (scalar queue: tiny, needed early) ----
        ident_f = glob.tile([128, 128], F32, tag="ident")
        make_identity(nc, ident_f)
        ones_bf = glob.tile([1, 512], BF, tag="ones_bf")
        nc.vector.memset(ones_bf, 1.0)
        ones64_b = glob.tile([1, 64], BF, tag="ones64")
        nc.vector.memset(ones64_b, 1.0)
        eps_t = glob.tile([128, 1], F32, tag="eps")
        nc.vector.memset(eps_t, EPS)
        nshift_t = glob.tile([128, 1], F32, tag="nshift")
        nc.vector.memset(nshift_t, -SHIFT)
        zero_t = glob.tile([128, 1], F32, tag="zero")
        nc.vector.memset(zero_t, 0.0)

        biasf_sb = glob.tile([128, KD + KF], F32, tag="biasf")
        nc.scalar.dma_start(out=biasf_sb, in_=io["bias_f"])
        bq8_c = biasf_sb[:, 0:KD]
        b1_c = biasf_sb[:, KD : KD + KF]

        biasb_sb = glob.tile([1, D], BF, tag="biasb")
        nc.scalar.dma_start(out=biasb_sb, in_=io["bias_b"])
        b2_r = biasb_sb[:, 0:D]

        mrow_sb = glob.tile([128, MS], F32, tag="mrow")
        nc.scalar.dma_start(out=mrow_sb, in_=io["mrow"])

        gbt = glob.tile([128, 4, D], BF, tag="gbt")
        g1b, be1b, g2b, be2b = gbt[:, 0, :], gbt[:, 1, :], gbt[:, 2, :], gbt[:, 3, :]

        w1_sb = glob.tile([128, KD, Fd], BF, tag="w1")
        h1_sb = glob.tile([128, MS, D], F32, tag="h1")
        h1T_sb = glob.tile([128, KD, S], BF, tag="h1T")
        x_sb = glob.tile([128, MS, D], BF, tag="x")

        # ================= attention scope =================
        with tc.tile_pool(name="attn", bufs=1) as attnp:
            xT_sb = attnp.tile([128, KD, S], F8, tag="xT")
            # th-slice 0 as separate tiles: tile-granular dependency tracking
            # means the first qk matmuls would otherwise wait for the whole
            # wq/wk transfer.
            wq0_sb = attnp.tile([128, 1, KD, 128], F8, tag="wq0")
            wk0_sb = attnp.tile([128, 1, KD, 128], F8, tag="wk0")
            wqr_sb = attnp.tile([128, 5, KD, 128], F8, tag="wqr")
            wkr_sb = attnp.tile([128, 5, KD, 128], F8, tag="wkr")
            wv_sb = attnp.tile([128, KD, D], F8, tag="wv")
            wp_sb = attnp.tile([128, KD, D], F8, tag="wp")
            gband_sb = attnp.tile([128, 2, 2, BAND], BF, tag="gband")

            def wq_sl(th):
                return (wq0_sb if th == 0 else wqr_sb)[:, 0 if th == 0 else th - 1]

            def wk_sl(th):
                return (wk0_sb if th == 0 else wkr_sb)[:, 0 if th == 0 else th - 1]

            # big-lead DMAs: descriptor gen costs ~0.65us per trigger, so
            # order by first use: xT+wq0+wk0 (first matmuls), wv (v tiles),
            # gband (first exp), then the bulk qk weights.
            nc.gpsimd.dma_start(
                out=xT_sb.rearrange("p c s -> p (c s)"), in_=io["xT8"]
            )
            nc.gpsimd.dma_start(
                out=wq0_sb.rearrange("p t c n -> p (t c n)"), in_=io["wq8"][:, 0:D]
            )
            nc.gpsimd.dma_start(
                out=wk0_sb.rearrange("p t c n -> p (t c n)"), in_=io["wk8"][:, 0:D]
            )
            nc.gpsimd.dma_start(out=wv_sb.rearrange("p c n -> p (c n)"), in_=io["wv8"])
            nc.gpsimd.dma_start(
                out=gband_sb.rearrange("p a b w -> p (a b w)"), in_=io["gband"]
            )
            nc.gpsimd.dma_start(
                out=wqr_sb.rearrange("p t c n -> p (t c n)"),
                in_=io["wq8"][:, D : 6 * D],
            )
            nc.gpsimd.dma_start(
                out=wkr_sb.rearrange("p t c n -> p (t c n)"),
                in_=io["wk8"][:, D : 6 * D],
            )
            late_dmas = [
                nc.sync.dma_start(out=wp_sb.rearrange("p c n -> p (c n)"), in_=io["wp8"]),
                nc.sync.dma_start(out=gbt.rearrange("p c n -> p (c n)"), in_=io["gb"]),
                nc.sync.dma_start(out=x_sb.rearrange("p c n -> p (c n)"), in_=io["x"]),
            ]

            qT_sb = attnp.tile([128, KD, S], BF, tag="qT")
            kT_sb = attnp.tile([128, KD, S], BF, tag="kT")
            # 128-wide per-head slots: v | mask-col x64. The 64 replicated
            # mask columns make the attn@v matmul emit the softmax sums
            # broadcast across partitions 64..127 for free (PE cost depends
            # only on N), so normalization needs no separate broadcast.
            v_sb = attnp.tile([128, MS, H, 128], F8, tag="v")
            nc.gpsimd.memset(v_sb[:, :, :, Dh:128], 1.0)
            hT_sb = attnp.tile([128, KD, S], F8, tag="hT")

            with tc.tile_pool(name="psA", bufs=1, space="PSUM") as psA:

                def qk_tile(th):
                    # bk dropped: a per-query score shift cancels in softmax.
                    act = None
                    for w_sl, dst in ((wq_sl(th), qT_sb), (wk_sl(th), kT_sb)):
                        ps = psA.tile([128, 512], F32, tag="acc", bufs=2, name="ps_qk")
                        for j in range(KP):
                            nc.tensor.matmul(
                                ps, w_sl[:, 2 * j : 2 * j + 2, :],
                                xT_sb[:, 2 * j : 2 * j + 2, :],
                                start=(j == 0), stop=(j == KP - 1), perf_mode=DR,
                            )
                        if dst is qT_sb:
                            act = nc.scalar.activation(
                                out=dst[:, th, :], in_=ps, func=AF.Identity,
                                bias=bq8_c[:, th : th + 1], scale=0.125,
                            )
                        else:
                            act = nc.vector.tensor_copy(out=dst[:, th, :], in_=ps)
                    return act

                def v_tiles(n):
                    # bv host-folded into x; mask folded into v rows (and the
                    # sum column) so the softmax input needs no mask at all.
                    for c in range(MS):
                        ps = psA.tile([128, ND], F32, tag="acc", bufs=2, name="ps_v")
                        for j in range(KP):
                            nc.tensor.matmul(
                                ps, xT_sb[:, 2 * j : 2 * j + 2, 128 * c : 128 * (c + 1)],
                                wv_sb[:, 2 * j : 2 * j + 2, ND * n : ND * (n + 1)],
                                start=(j == 0), stop=(j == KP - 1), perf_mode=DR,
                            )
                        nc.vector.tensor_scalar(
                            out=v_sb[:, c, 6 * n : 6 * (n + 1), 0:Dh],
                            in0=ps.rearrange("p (h d) -> p h d", d=Dh),
                            scalar1=mrow_sb[:, c : c + 1], scalar2=None,
                            op0=OP.mult,
                        )
                    if n == 1:
                        for c in range(MS):
                            nc.vector.tensor_scalar(
                                out=v_sb[:, c, :, Dh:128],
                                in0=v_sb[:, c, :, Dh:128],
                                scalar1=mrow_sb[:, c : c + 1],
                                scalar2=None, op0=OP.mult,
                            )

                def scores_exp(h):
                    th, off = h // 2, (h % 2) * 64
                    qh = qT_sb[off : off + 64, th, :]
                    kh = kT_sb[off : off + 64, th, :]
                    exp_tiles = []
                    for half in range(2):
                        ps_sc = psA.tile([128, 2, 512], F32, tag="sc2", bufs=2, name="ps_sc")
                        for j in range(2):
                            c = 2 * half + j
                            nc.tensor.matmul(
                                ps_sc[:, j, :], kh[:, 128 * c : 128 * (c + 1)], qh,
                                start=True, stop=True,
                            )
                        ex = attnp.tile([128, 2, 512], F8, tag="exp", bufs=8, name="ex")
                        nc.scalar.activation(out=ex, in_=ps_sc, func=AF.Exp, bias=nshift_t)
                        # gauss factor: identity outside a 264-wide diagonal band
                        lo = 256 * half - 4
                        wlo, whi = max(0, -lo), min(BAND, 512 - lo)
                        eng = nc.gpsimd if half == 1 else nc.vector
                        eng.tensor_tensor(
                            out=ex[:, :, lo + wlo : lo + whi],
                            in0=ex[:, :, lo + wlo : lo + whi],
                            in1=gband_sb[:, half, :, wlo:whi], op=OP.mult,
                        )
                        exp_tiles.append(ex)
                    return exp_tiles

                def h_chain(h, exp_tiles):
                    ps_h = psA.tile([128, 512], F32, tag="hT", bufs=2, name="ps_h")
                    for half in range(2):
                        nc.tensor.matmul(
                            ps_h, v_sb[:, 2 * half : 2 * half + 2, h, :],
                            exp_tiles[half],
                            start=(half == 0), stop=(half == 1), perf_mode=DR,
                        )
                    # sums (rows 64..127, already partition-broadcast by the
                    # matmul) to SBUF, then a [64,512] reciprocal: approx
                    # cost is partition-blind, so this is as cheap as [1,512].
                    sums = attnp.tile([64, 512], F32, tag="sums", bufs=2, name="sums")
                    nc.scalar.copy(sums, ps_h[Dh:128, :])
                    rec = attnp.tile([64, 512], F32, tag="rec", bufs=2, name="rec")
                    nc.vector.reciprocal_approx_fast(out=rec, in_=sums)
                    return ps_h, rec

                def h_apply(h, ps_h, rec):
                    th, off = h // 2, (h % 2) * 64
                    return nc.vector.tensor_tensor(
                        out=hT_sb[off : off + 64, th, :], in0=ps_h[0:Dh, :],
                        in1=rec, op=OP.mult,
                    )

                # software-pipelined head loop: per iteration each engine
                # sees at most one stage of each kind, so stage latencies of
                # consecutive heads overlap instead of serializing.
                qk_acts = [qk_tile(0), qk_tile(1)]
                # not needed until proj/LN1: keep them off the DMA rings
                # while the attention lead-in streams in
                for dma in late_dmas:
                    add_dep_helper(dma.ins, qk_acts[0].ins, True, "defer until lead-in clear")
                exps, chains = {}, {}
                exps[0] = scores_exp(0)
                v_tiles(0)
                exps[1] = scores_exp(1)
                v_tiles(1)
                last_mult = None
                for h in range(H + 2):
                    if h < H and h not in exps:
                        exps[h] = scores_exp(h)
                        if 2 <= h <= 5:
                            qk_acts.append(qk_tile(h))
                    if 1 <= h <= H:
                        chains[h - 1] = h_chain(h - 1, exps.pop(h - 1))
                    if h >= 2:
                        last_mult = h_apply(h - 2, *chains.pop(h - 2))

                # defer the big FFN weight DMAs until the lead-in is clear;
                # rings are idle mid-attention, so anchor on the 3rd qk tile
                w1dma = nc.sync.dma_start(out=w1_sb.rearrange("p c n -> p (c n)"), in_=io["w1_bf"])
                add_dep_helper(w1dma.ins, qk_acts[2].ins, True, "defer w1 until lead-in clear")

                # --- proj (fp8 DoubleRow) + residual + LN1 + h1 transpose,
                # software-pipelined: transposes of chunk m-1 fill the PE
                # while chunk m's LN chain runs on DVE/ACT ---
                def proj_chunk(m):
                    pss = []
                    for n in range(NT):
                        ps = psA.tile([128, ND], F32, tag="acc", bufs=2, name="ps_pr")
                        for j in range(KP):
                            nc.tensor.matmul(
                                ps, hT_sb[:, 2 * j : 2 * j + 2, 128 * m : 128 * (m + 1)],
                                wp_sb[:, 2 * j : 2 * j + 2, ND * n : ND * (n + 1)],
                                start=(j == 0), stop=(j == KP - 1), perf_mode=DR,
                            )
                        pss.append(ps)
                    row = glob.tile([128, D], F32, tag="rowtmp", bufs=2, name="row")
                    for n in range(NT):
                        nc.vector.tensor_tensor(
                            out=row[:, ND * n : ND * (n + 1)], in0=pss[n],
                            in1=x_sb[:, m, ND * n : ND * (n + 1)], op=OP.add,
                        )
                    _layernorm(nc, glob, row, g1b, be1b, eps_t, h1_sb[:, m, :])

                def transpose_chunk(m):
                    for f in range(KD):
                        ps_t = psA.tile([128, 128], F32, tag="acc", bufs=2, name="ps_t")
                        nc.tensor.transpose(ps_t, h1_sb[:, m, 128 * f : 128 * (f + 1)], ident_f)
                        nc.scalar.copy(out=h1T_sb[:, f, 128 * m : 128 * (m + 1)], in_=ps_t)

                proj_chunk(0)
                proj_chunk(1)
                transpose_chunk(0)
                proj_chunk(2)
                transpose_chunk(1)
                proj_chunk(3)
                transpose_chunk(2)
                transpose_chunk(3)

        # ================= FFN scope (bf16) =================
        with tc.tile_pool(name="ffn", bufs=1) as ffnp, \
             tc.tile_pool(name="psF", bufs=1, space="PSUM") as psF:
            ff1T_sb = ffnp.tile([128, KF, S], BF, tag="ff1T")
            w2_sb = ffnp.tile([128, KF, D], BF, tag="w2")
            w2dma = nc.sync.dma_start(out=w2_sb.rearrange("p c n -> p (c n)"), in_=io["w2_bf"])
            add_dep_helper(w2dma.ins, last_mult.ins, True, "defer w2 until attention done")
            # two half-S passes: the first needs only h1 chunks 0-1, so it
            # overlaps the LN1/transpose ramp of chunks 2-3.
            for half in range(2):
                sl = slice(256 * half, 256 * (half + 1))
                for fm in range(KF):
                    ps = psF.tile([128, 256], F32, tag="acc", bufs=8, name="ps_f1")
                    for k in range(KD):
                        nc.tensor.matmul(
                            ps, w1_sb[:, k, 128 * fm : 128 * (fm + 1)],
                            h1T_sb[:, k, sl],
                            start=(k == 0), stop=(k == KD - 1),
                        )
                    nc.scalar.activation(
                        out=ff1T_sb[:, fm, sl], in_=ps, func=AF.Gelu,
                        bias=b1_c[:, fm : fm + 1], scale=1.0,
                    )

            for m in range(MS):
                pss = []
                for n in range(NT):
                    ps = psF.tile([128, ND], F32, tag="acc", bufs=8, name="ps_f2")
                    for k in range(KF):
                        nc.tensor.matmul(
                            ps, ff1T_sb[:, k, 128 * m : 128 * (m + 1)],
                            w2_sb[:, k, ND * n : ND * (n + 1)],
                            start=(k == 0), stop=False,
                        )
                    nc.tensor.matmul(
                        ps, ones_bf[:, 0:128], b2_r[:, ND * n : ND * (n + 1)],
                        start=False, stop=True,
                    )
                    pss.append(ps)
                row = glob.tile([128, D], F32, tag="rowtmp", bufs=2, name="row2")
                for n in range(NT):
                    nc.vector.tensor_tensor(
                        out=row[:, ND * n : ND * (n + 1)], in0=pss[n],
                        in1=h1_sb[:, m, ND * n : ND * (n + 1)], op=OP.add,
                    )
                outrow = glob.tile([128, D], F32, tag="outrow", bufs=2, name="outrow")
                _layernorm(nc, glob, row, g2b, be2b, eps_t, outrow)
                nc.gpsimd.dma_start(
                    out=io["out"][128 * m : 128 * (m + 1), :], in_=outrow
                )


def _layernorm(nc, pool, row, gamma_b, beta_b, eps_t, out_ap):
    st = pool.tile([128, 3, 6], F32, tag="st", bufs=2, name="st")
    for g in range(3):
        nc.vector.bn_stats(out=st[:, g, :], in_=row[:, 256 * g : 256 * (g + 1)])
    mv = pool.tile([128, 2], F32, tag="mv", bufs=2, name="mv")
    nc.vector.bn_aggr(out=mv, in_=st)
    sd = pool.tile([128, 1], F32, tag="sd", bufs=2, name="sd")
    nc.scalar.activation(out=sd, in_=mv[:, 1:2], func=AF.Sqrt, bias=eps_t, scale=1.0)
    rs = pool.tile([128, 1], F32, tag="rs", bufs=2, name="rs")
    nc.vector.reciprocal(rs, sd)
    # in-place: row = (row - mean) * gamma ; out = row * rstd + beta
    nc.vector.scalar_tensor_tensor(
        out=row, in0=row, scalar=mv[:, 0:1], in1=gamma_b, op0=OP.subtract, op1=OP.mult
    )
    nc.vector.scalar_tensor_tensor(
        out=out_ap, in0=row, scalar=rs, in1=beta_b, op0=OP.mult, op1=OP.add
    )


_SPECS = [
    # (name, shape, dtype) - big tensors pre-permuted on host to SBUF layout
    ("x", [128, MS * D], BF),
    ("xT8", [128, KD * S], F8),
    ("wq8", [128, 6 * D], F8),       # th-major: [p, th, k, 128]
    ("wk8", [128, 6 * D], F8),
    ("wv8", [128, KD * D], F8),
    ("wp8", [128, KD * D], F8),
    ("gband", [128, 2 * 2 * BAND], BF),
    ("mrow", [128, MS], F32),
    ("w1_bf", [128, KD * Fd], BF),
    ("w2_bf", [128, KF * D], BF),
    ("bias_f", [128, KD + KF], F32),   # bq8 | b1, per-partition cols
    ("bias_b", [1, D], BF),            # b2 row
    ("gb", [128, 4 * D], BF),          # gamma1|beta1|gamma2|beta2 (host-bcast)
]

_BUILT = {}


def _build():
    if "nc" in _BUILT:
        return _BUILT["nc"]
    nc = bacc.Bacc("TRN2", target_bir_lowering=False, debug=False,
                   enable_asserts=False, num_devices=NCORES)
    io = {}
    for name, shape, dt in _SPECS:
        io[name] = nc.dram_tensor(name, shape, dt, kind="ExternalInput").ap()
    io["out"] = nc.dram_tensor("out", [S, D], F32, kind="ExternalOutput").ap()
    _trace(nc, io)
    nc.compile()
    _BUILT["nc"] = nc
    return nc


def _host_prep(inputs):
    bf = ml_dtypes.bfloat16
    f8 = ml_dtypes.float8_e4m3
    f32 = np.float32
    x = np.asarray(inputs["x"], f32)
    mask = np.asarray(inputs["mask"])

    def q8(a):
        return np.asarray(np.clip(a, -240.0, 240.0), f8)

    idx = np.arange(S, dtype=np.float64)
    dd = idx[None, :] - idx[:, None]
    sc = -0.5 * dd * dd
    sc -= sc.max(axis=-1, keepdims=True)
    e = np.exp(sc)
    gauss = e / e.sum(axis=-1, keepdims=True)  # [query s, key t], float64

    # band factor table: gband[t_loc, half, j, w] = exp(gauss[s, t]),
    # with t = 256*half + 128*j + t_loc, s = 256*half - 4 + w (1.0 if s OOB)
    t_loc = np.arange(128)
    gband = np.ones((128, 2, 2, BAND), np.float64)
    for half in range(2):
        for j in range(2):
            t_abs = 256 * half + 128 * j + t_loc          # [128]
            s_abs = 256 * half - 4 + np.arange(BAND)      # [BAND]
            valid = (s_abs >= 0) & (s_abs < S)
            gband[:, half, j, valid] = np.exp(gauss[s_abs[valid][None, :], t_abs[:, None]])
    gband = np.ascontiguousarray(gband.reshape(128, -1)).astype(bf)

    def sbl(a, p=128):  # [C*p, N] -> [p, C*N] (SBUF layout)
        cN = a.shape[0] // p
        return np.ascontiguousarray(
            a.reshape(cN, p, a.shape[1]).transpose(1, 0, 2).reshape(p, -1)
        )

    def thmaj(w):  # [D, D] -> [128, th, k, 128] flat (th-major fp8)
        a = np.asarray(w, f32).reshape(KD, 128, KD, 128)
        return np.ascontiguousarray(
            q8(a.transpose(1, 2, 0, 3)).reshape(128, -1)
        )

    def pcols(a, p=128):  # [C*p] -> [p, C] per-partition columns
        return np.ascontiguousarray(a.reshape(-1, p).T)

    bias_f = np.concatenate(
        [
            pcols(np.asarray(inputs["bq"], f32) * np.float32(0.125)),
            pcols(np.asarray(inputs["b1"], f32)),
        ],
        axis=1,
    )
    bias_b = np.asarray(inputs["b2"], f32).astype(bf)[None, :]
    # v/proj biases fold into the residual: x + h@wproj + bproj + bv@wproj
    x_fold = (
        np.asarray(inputs["b_proj"], f32)
        + np.asarray(inputs["bv"], f32) @ np.asarray(inputs["w_proj"], f32)
    )
    gb = np.ascontiguousarray(
        np.broadcast_to(
            np.concatenate(
                [
                    np.asarray(inputs["gamma1"], f32),
                    np.asarray(inputs["beta1"], f32),
                    np.asarray(inputs["gamma2"], f32),
                    np.asarray(inputs["beta2"], f32),
                ]
            )[None, :],
            (128, 4 * D),
        )
    ).astype(bf)
    shared = {
        "wq8": thmaj(inputs["wq"]),
        "wk8": thmaj(inputs["wk"]),
        "wv8": sbl(q8(np.asarray(inputs["wv"], f32))),
        "wp8": sbl(q8(np.asarray(inputs["w_proj"], f32))),
        "w1_bf": sbl(np.asarray(inputs["w1"], f32).astype(bf)),
        "w2_bf": sbl(np.asarray(inputs["w2"], f32).astype(bf)),
        "gband": gband,
        "bias_f": bias_f,
        "bias_b": bias_b,
        "gb": gb,
    }
    in_maps = []
    for b in range(NCORES):
        m = dict(shared)
        m["x"] = sbl(np.ascontiguousarray(x[b] + x_fold[None, :]).astype(bf))
        m["xT8"] = sbl(q8(np.ascontiguousarray(x[b].T)))
        m["mrow"] = np.ascontiguousarray(
            mask[b].astype(f32).reshape(MS, 128).T
        )
        in_maps.append(m)
    return in_maps


def _run(inputs, trace=False, trace_cores=None):
    nc = _build()
    in_maps = _host_prep(inputs)
    res = bass_utils.run_bass_kernel_spmd(
        nc, in_maps, core_ids=list(range(NCORES)), trace=trace,
        trace_cores=trace_cores,
    )
    out = np.stack([np.asarray(res.results[b]["out"]) for b in range(NCORES)])
    return out.astype(np.float32), res


def kernel(**inputs):
    return _run(inputs)[0]


# revision 63
# speedup vs baseline: 1.1927x; 1.0028x over previous
"""Trainium2 Bass kernel for a dense transformer block (B=8, S=512, D=768, H=12, Fd=3072).

Sharding: pure data-parallel over batch - one batch element per NeuronCore,
weights replicated, no collectives.

Key structure (v2):
  - Attention projections (q,k,v), the attn@v contraction and the output
    projection run in fp8e4m3 with DoubleRow perf mode (2 k-tiles per pass,
    2x PE throughput).  Scores (K=64) and the FFN stay bf16: fp8 would
    blow the 2e-2 error budget (measured 2.2e-2 for fp8 FFN in simulation),
    while qkv+attn+proj fp8 sims at 1.15e-2 total.
  - scoresT[t,s] layout (keys on partitions) so the key-padding mask is a
    per-partition factor: it folds into v (v rows and the sum-column get
    multiplied by mask), not into the softmax input.
  - The Gaussian positional bias enters as a multiplicative factor
    exp(gauss[s,t]) on exp(scores).  Since gauss < 2.3e-6 beyond |t-s| >= 4,
    the factor is 1.0 outside a 264-wide diagonal band: one small in-place
    band multiply per score tile instead of a full [512,512] multiply.
  - exp is computed with a fixed -3.5 shift (ACT bias) so values fit fp8
    range (max ~80 < 240); the shift cancels in the softmax normalization.
  - softmax sums come from a mask-valued extra column appended to v; the
    per-column reciprocal is broadcast across partitions with a 1-row
    f32r matmul (f32r: 1 cycle/row at N=512, no cast needed).
  - v/proj biases fold into the residual input on the host:
    x + b_proj + bv @ w_proj.  bk drops entirely (a per-query score shift
    cancels in softmax).  bq rides the qT activation bias, b1 the gelu
    bias, b2 a K=1 ones-row matmul.
  - Big weight DMAs are triggered from the Pool (gpsimd) queue (25ns per
    trigger vs 667ns on ACT/DVE), ordered so the first qk matmul only
    waits for xT + the first th-slice of wq.
"""

import numpy as np
import ml_dtypes

import concourse.bass as bass
import concourse.mybir as mybir
import concourse.tile as tile
from concourse.tile import add_dep_helper
from concourse import bacc
from concourse import bass_utils
from concourse.masks import make_identity

BF = mybir.dt.bfloat16
F32 = mybir.dt.float32
F32R = mybir.dt.float32r
F8 = mybir.dt.float8e4
AF = mybir.ActivationFunctionType
OP = mybir.AluOpType
DR = mybir.MatmulPerfMode.DoubleRow

B, S, D, H, Dh, Fd = 8, 512, 768, 12, 64, 3072
NCORES = 8
EPS = 1e-12
SHIFT = 3.5        # exp(score - SHIFT): keeps fp8 exp values < 240
BAND = 264         # gauss factor band width (|t-s|<=4 significant)

KD = D // 128      # 6  K-tiles over D
KP = KD // 2       # 3  K-tile pairs (DoubleRow)
MS = S // 128      # 4  chunks over sequence
KF = Fd // 128     # 24 K-tiles over Fd
NT = 2             # N-tiles over D for natural-layout outputs (2 x 384)
ND = D // NT       # 384


def _trace(nc, io):
    with tile.TileContext(nc) as tc:
        _trace_body(nc, tc, io)


def _act_recip(nc, out, in_):
    """ACT-engine reciprocal (InstActivation, func=Reciprocal).

    The bass wrapper refuses Reciprocal on ACT for accuracy reasons; the
    softmax normalizer only needs ~1e-2 relative accuracy, which the ACT
    table comfortably provides (validated against the reference output).
    """
    eng = nc.scalar
    inputs = [eng.lower_ap(in_)]
    for val in (0.0, 1.0, 0.0):  # bias, scale, alpha immediates
        inputs.append(mybir.ImmediateValue(dtype=mybir.dt.float32, value=val))
    return eng.add_instruction(
        mybir.InstActivation(
            name=eng.bass.get_next_instruction_name(),
            func=AF.Reciprocal,
            ins=inputs,
            outs=[eng.lower_ap(out)],
        )
    )


def _trace_body(nc, tc, io):
    from contextlib import ExitStack

    with ExitStack() as ctx:
        glob = ctx.enter_context(tc.tile_pool(name="glob", bufs=1))

        # ---- constants / small inputs (scalar queue: tiny, needed early) ----
        ident_f = glob.tile([128, 128], F32, tag="ident")
        make_identity(nc, ident_f)
        ones_bf = glob.tile([1, 512], BF, tag="ones_bf")
        nc.vector.memset(ones_bf, 1.0)
        ones64_b = glob.tile([1, 64], BF, tag="ones64")
        nc.vector.memset(ones64_b, 1.0)
        eps_t = glob.tile([128, 1], F32, tag="eps")
        nc.vector.memset(eps_t, EPS)
        nshift_t = glob.tile([128, 1], F32, tag="nshift")
        nc.vector.memset(nshift_t, -SHIFT)
        zero_t = glob.tile([128, 1], F32, tag="zero")
        nc.vector.memset(zero_t, 0.0)

        biasf_sb = glob.tile([128, KD + KF], F32, tag="biasf")
        nc.scalar.dma_start(out=biasf_sb, in_=io["bias_f"])
        bq8_c = biasf_sb[:, 0:KD]
        b1_c = biasf_sb[:, KD : KD + KF]

        biasb_sb = glob.tile([1, D], BF, tag="biasb")
        nc.scalar.dma_start(out=biasb_sb, in_=io["bias_b"])
        b2_r = biasb_sb[:, 0:D]

        mrow_sb = glob.tile([128, MS], F32, tag="mrow")
        nc.scalar.dma_start(out=mrow_sb, in_=io["mrow"])

        gbt = glob.tile([128, 4, D], BF, tag="gbt")
        g1b, be1b, g2b, be2b = gbt[:, 0, :], gbt[:, 1, :], gbt[:, 2, :], gbt[:, 3, :]

        w1_sb = glob.tile([128, KD, Fd], BF, tag="w1")
        h1_sb = glob.tile([128, MS, D], F32, tag="h1")
        h1T_sb = glob.tile([128, KD, S], BF, tag="h1T")
        x_sb = glob.tile([128, MS, D], BF, tag="x")

        # ================= attention scope =================
        with tc.tile_pool(name="attn", bufs=1) as attnp:
            xT_sb = attnp.tile([128, KD, S], F8, tag="xT")
            # th-slice 0 as separate tiles: tile-granular dependency tracking
            # means the first qk matmuls would otherwise wait for the whole
            # wq/wk transfer.
            wq0_sb = attnp.tile([128, 1, KD, 128], F8, tag="wq0")
            wk0_sb = attnp.tile([128, 1, KD, 128], F8, tag="wk0")
            wqr_sb = attnp.tile([128, 5, KD, 128], F8, tag="wqr")
            wkr_sb = attnp.tile([128, 5, KD, 128], F8, tag="wkr")
            wv_sb = attnp.tile([128, KD, D], F8, tag="wv")
            wp_sb = attnp.tile([128, KD, D], F8, tag="wp")
            gband_sb = attnp.tile([128, 2, 2, BAND], BF, tag="gband")

            def wq_sl(th):
                return (wq0_sb if th == 0 else wqr_sb)[:, 0 if th == 0 else th - 1]

            def wk_sl(th):
                return (wk0_sb if th == 0 else wkr_sb)[:, 0 if th == 0 else th - 1]

            # big-lead DMAs: descriptor gen costs ~0.65us per trigger, so
            # order by first use: xT+wq0+wk0 (first matmuls), wv (v tiles),
            # gband (first exp), then the bulk qk weights.
            nc.gpsimd.dma_start(
                out=xT_sb.rearrange("p c s -> p (c s)"), in_=io["xT8"]
            )
            nc.gpsimd.dma_start(
                out=wq0_sb.rearrange("p t c n -> p (t c n)"), in_=io["wq8"][:, 0:D]
            )
            nc.gpsimd.dma_start(
                out=wk0_sb.rearrange("p t c n -> p (t c n)"), in_=io["wk8"][:, 0:D]
            )
            nc.gpsimd.dma_start(out=wv_sb.rearrange("p c n -> p (c n)"), in_=io["wv8"])
            nc.gpsimd.dma_start(
                out=gband_sb.rearrange("p a b w -> p (a b w)"), in_=io["gband"]
            )
            nc.gpsimd.dma_start(
                out=wqr_sb.rearrange("p t c n -> p (t c n)"),
                in_=io["wq8"][:, D : 6 * D],
            )
            nc.gpsimd.dma_start(
                out=wkr_sb.rearrange("p t c n -> p (t c n)"),
                in_=io["wk8"][:, D : 6 * D],
            )
            late_dmas = [
                nc.sync.dma_start(out=wp_sb.rearrange("p c n -> p (c n)"), in_=io["wp8"]),
                nc.sync.dma_start(out=gbt.rearrange("p c n -> p (c n)"), in_=io["gb"]),
                nc.sync.dma_start(out=x_sb.rearrange("p c n -> p (c n)"), in_=io["x"]),
            ]

            qT_sb = attnp.tile([128, KD, S], BF, tag="qT")
            kT_sb = attnp.tile([128, KD, S], BF, tag="kT")
            # 128-wide per-head slots: v | mask-col x64. The 64 replicated
            # mask columns make the attn@v matmul emit the softmax sums
            # broadcast across partitions 64..127 for free (PE cost depends
            # only on N), so normalization needs no separate broadcast.
            v_sb = attnp.tile([128, MS, H, 128], F8, tag="v")
            nc.gpsimd.memset(v_sb[:, :, :, Dh:128], 1.0)
            hT_sb = attnp.tile([128, KD, S], F8, tag="hT")

            with tc.tile_pool(name="psA", bufs=1, space="PSUM") as psA:

                def qk_tile(th):
                    # bk dropped: a per-query score shift cancels in softmax.
                    act = None
                    for w_sl, dst in ((wq_sl(th), qT_sb), (wk_sl(th), kT_sb)):
                        ps = psA.tile([128, 512], F32, tag="acc", bufs=2, name="ps_qk")
                        for j in range(KP):
                            nc.tensor.matmul(
                                ps, w_sl[:, 2 * j : 2 * j + 2, :],
                                xT_sb[:, 2 * j : 2 * j + 2, :],
                                start=(j == 0), stop=(j == KP - 1), perf_mode=DR,
                            )
                        if dst is qT_sb:
                            act = nc.scalar.activation(
                                out=dst[:, th, :], in_=ps, func=AF.Identity,
                                bias=bq8_c[:, th : th + 1], scale=0.125,
                            )
                        else:
                            act = nc.vector.tensor_copy(out=dst[:, th, :], in_=ps)
                    return act

                def v_tiles(n):
                    # bv host-folded into x; mask folded into v rows (and the
                    # sum column) so the softmax input needs no mask at all.
                    for c in range(MS):
                        ps = psA.tile([128, ND], F32, tag="acc", bufs=2, name="ps_v")
                        for j in range(KP):
                            nc.tensor.matmul(
                                ps, xT_sb[:, 2 * j : 2 * j + 2, 128 * c : 128 * (c + 1)],
                                wv_sb[:, 2 * j : 2 * j + 2, ND * n : ND * (n + 1)],
                                start=(j == 0), stop=(j == KP - 1), perf_mode=DR,
                            )
                        nc.vector.tensor_scalar(
                            out=v_sb[:, c, 6 * n : 6 * (n + 1), 0:Dh],
                            in0=ps.rearrange("p (h d) -> p h d", d=Dh),
                            scalar1=mrow_sb[:, c : c + 1], scalar2=None,
                            op0=OP.mult,
                        )
                    if n == 1:
                        for c in range(MS):
                            nc.vector.tensor_scalar(
                                out=v_sb[:, c, :, Dh:128],
                                in0=v_sb[:, c, :, Dh:128],
                                scalar1=mrow_sb[:, c : c + 1],
                                scalar2=None, op0=OP.mult,
                            )

                def scores_exp(h):
                    th, off = h // 2, (h % 2) * 64
                    qh = qT_sb[off : off + 64, th, :]
                    kh = kT_sb[off : off + 64, th, :]
                    exp_tiles = []
                    for half in range(2):
                        ex = attnp.tile([128, 2, 512], F8, tag="exp", bufs=8, name="ex")
                        for j in range(2):
                            c = 2 * half + j
                            ps_sc = psA.tile([128, 512], F32, tag="sc2", bufs=4, name="ps_sc")
                            nc.tensor.matmul(
                                ps_sc, kh[:, 128 * c : 128 * (c + 1)], qh,
                                start=True, stop=True,
                            )
                            nc.scalar.activation(
                                out=ex[:, j, :], in_=ps_sc, func=AF.Exp, bias=nshift_t
                            )
                        # gauss factor: identity outside a 264-wide diagonal band
                        lo = 256 * half - 4
                        wlo, whi = max(0, -lo), min(BAND, 512 - lo)
                        eng = nc.gpsimd if half == 1 else nc.vector
                        eng.tensor_tensor(
                            out=ex[:, :, lo + wlo : lo + whi],
                            in0=ex[:, :, lo + wlo : lo + whi],
                            in1=gband_sb[:, half, :, wlo:whi], op=OP.mult,
                        )
                        exp_tiles.append(ex)
                    return exp_tiles

                def h_chain(h, exp_tiles):
                    ps_h = psA.tile([128, 512], F32, tag="hT", bufs=2, name="ps_h")
                    for half in range(2):
                        nc.tensor.matmul(
                            ps_h, v_sb[:, 2 * half : 2 * half + 2, h, :],
                            exp_tiles[half],
                            start=(half == 0), stop=(half == 1), perf_mode=DR,
                        )
                    # sums (rows 64..127, already partition-broadcast by the
                    # matmul) to SBUF, then a [64,512] reciprocal: approx
                    # cost is partition-blind, so this is as cheap as [1,512].
                    sums = attnp.tile([64, 512], F32, tag="sums", bufs=2, name="sums")
                    nc.scalar.copy(sums, ps_h[Dh:128, :])
                    rec = attnp.tile([64, 512], F32, tag="rec", bufs=2, name="rec")
                    nc.vector.reciprocal_approx_fast(out=rec, in_=sums)
                    return ps_h, rec

                def h_apply(h, ps_h, rec):
                    th, off = h // 2, (h % 2) * 64
                    return nc.vector.tensor_tensor(
                        out=hT_sb[off : off + 64, th, :], in0=ps_h[0:Dh, :],
                        in1=rec, op=OP.mult,
                    )

                # software-pipelined head loop: per iteration each engine
                # sees at most one stage of each kind, so stage latencies of
                # consecutive heads overlap instead of serializing.
                qk_acts = [qk_tile(0), qk_tile(1)]
                # not needed until proj/LN1: keep them off the DMA rings
                # while the attention lead-in streams in
                for dma in late_dmas:
                    add_dep_helper(dma.ins, qk_acts[0].ins, True, "defer until lead-in clear")
                exps, chains = {}, {}
                exps[0] = scores_exp(0)
                v_tiles(0)
                exps[1] = scores_exp(1)
                v_tiles(1)
                last_mult = None
                for h in range(H + 2):
                    if h < H and h not in exps:
                        exps[h] = scores_exp(h)
                        if 2 <= h <= 5:
                            qk_acts.append(qk_tile(h))
                    if 1 <= h <= H:
                        chains[h - 1] = h_chain(h - 1, exps.pop(h - 1))
                    if h >= 2:
                        last_mult = h_apply(h - 2, *chains.pop(h - 2))

                # defer the big FFN weight DMAs until the lead-in is clear;
                # rings are idle mid-attention, so anchor on the 3rd qk tile
                w1dma = nc.sync.dma_start(out=w1_sb.rearrange("p c n -> p (c n)"), in_=io["w1_bf"])
                add_dep_helper(w1dma.ins, qk_acts[2].ins, True, "defer w1 until lead-in clear")

                # --- proj (fp8 DoubleRow) + residual + LN1 + h1 transpose,
                # software-pipelined: transposes of chunk m-1 fill the PE
                # while chunk m's LN chain runs on DVE/ACT ---
                def proj_chunk(m):
                    pss = []
                    for n in range(NT):
                        ps = psA.tile([128, ND], F32, tag="acc", bufs=2, name="ps_pr")
                        for j in range(KP):
                            nc.tensor.matmul(
                                ps, hT_sb[:, 2 * j : 2 * j + 2, 128 * m : 128 * (m + 1)],
                                wp_sb[:, 2 * j : 2 * j + 2, ND * n : ND * (n + 1)],
                                start=(j == 0), stop=(j == KP - 1), perf_mode=DR,
                            )
                        pss.append(ps)
                    row = glob.tile([128, D], F32, tag="rowtmp", bufs=2, name="row")
                    for n in range(NT):
                        nc.vector.tensor_tensor(
                            out=row[:, ND * n : ND * (n + 1)], in0=pss[n],
                            in1=x_sb[:, m, ND * n : ND * (n + 1)], op=OP.add,
                        )
                    _layernorm(nc, glob, row, g1b, be1b, eps_t, h1_sb[:, m, :])

                def transpose_chunk(m):
                    for f in range(KD):
                        ps_t = psA.tile([128, 128], F32, tag="acc", bufs=2, name="ps_t")
                        nc.tensor.transpose(ps_t, h1_sb[:, m, 128 * f : 128 * (f + 1)], ident_f)
                        nc.scalar.copy(out=h1T_sb[:, f, 128 * m : 128 * (m + 1)], in_=ps_t)

                proj_chunk(0)
                proj_chunk(1)
                transpose_chunk(0)
                proj_chunk(2)
                transpose_chunk(1)
                proj_chunk(3)
                transpose_chunk(2)
                transpose_chunk(3)

        # ================= FFN scope (bf16) =================
        with tc.tile_pool(name="ffn", bufs=1) as ffnp, \
             tc.tile_pool(name="psF", bufs=1, space="PSUM") as psF:
            ff1T_sb = ffnp.tile([128, KF, S], BF, tag="ff1T")
            w2_sb = ffnp.tile([128, KF, D], BF, tag="w2")
            w2dma = nc.sync.dma_start(out=w2_sb.rearrange("p c n -> p (c n)"), in_=io["w2_bf"])
            add_dep_helper(w2dma.ins, last_mult.ins, True, "defer w2 until attention done")
            # two half-S passes: the first needs only h1 chunks 0-1, so it
            # overlaps the LN1/transpose ramp of chunks 2-3.
            for half in range(2):
                sl = slice(256 * half, 256 * (half + 1))
                for fm in range(KF):
                    ps = psF.tile([128, 256], F32, tag="acc", bufs=8, name="ps_f1")
                    for k in range(KD):
                        nc.tensor.matmul(
                            ps, w1_sb[:, k, 128 * fm : 128 * (fm + 1)],
                            h1T_sb[:, k, sl],
                            start=(k == 0), stop=(k == KD - 1),
                        )
                    nc.scalar.activation(
                        out=ff1T_sb[:, fm, sl], in_=ps, func=AF.Gelu,
                        bias=b1_c[:, fm : fm + 1], scale=1.0,
                    )

            for m in range(MS):
                pss = []
                for n in range(NT):
                    ps = psF.tile([128, ND], F32, tag="acc", bufs=8, name="ps_f2")
                    for k in range(KF):
                        nc.tensor.matmul(
                            ps, ff1T_sb[:, k, 128 * m : 128 * (m + 1)],
                            w2_sb[:, k, ND * n : ND * (n + 1)],
                            start=(k == 0), stop=False,
                        )
                    nc.tensor.matmul(
                        ps, ones_bf[:, 0:128], b2_r[:, ND * n : ND * (n + 1)],
                        start=False, stop=True,
                    )
                    pss.append(ps)
                row = glob.tile([128, D], F32, tag="rowtmp", bufs=2, name="row2")
                for n in range(NT):
                    nc.vector.tensor_tensor(
                        out=row[:, ND * n : ND * (n + 1)], in0=pss[n],
                        in1=h1_sb[:, m, ND * n : ND * (n + 1)], op=OP.add,
                    )
                outrow = glob.tile([128, D], F32, tag="outrow", bufs=2, name="outrow")
                _layernorm(nc, glob, row, g2b, be2b, eps_t, outrow)
                nc.gpsimd.dma_start(
                    out=io["out"][128 * m : 128 * (m + 1), :], in_=outrow
                )


def _layernorm(nc, pool, row, gamma_b, beta_b, eps_t, out_ap):
    st = pool.tile([128, 3, 6], F32, tag="st", bufs=2, name="st")
    for g in range(3):
        nc.vector.bn_stats(out=st[:, g, :], in_=row[:, 256 * g : 256 * (g + 1)])
    mv = pool.tile([128, 2], F32, tag="mv", bufs=2, name="mv")
    nc.vector.bn_aggr(out=mv, in_=st)
    sd = pool.tile([128, 1], F32, tag="sd", bufs=2, name="sd")
    nc.scalar.activation(out=sd, in_=mv[:, 1:2], func=AF.Sqrt, bias=eps_t, scale=1.0)
    rs = pool.tile([128, 1], F32, tag="rs", bufs=2, name="rs")
    nc.vector.reciprocal(rs, sd)
    # in-place: row = (row - mean) * gamma ; out = row * rstd + beta
    nc.vector.scalar_tensor_tensor(
        out=row, in0=row, scalar=mv[:, 0:1], in1=gamma_b, op0=OP.subtract, op1=OP.mult
    )
    nc.vector.scalar_tensor_tensor(
        out=out_ap, in0=row, scalar=rs, in1=beta_b, op0=OP.mult, op1=OP.add
    )


_SPECS = [
    # (name, shape, dtype) - big tensors pre-permuted on host to SBUF layout
    ("x", [128, MS * D], BF),
    ("xT8", [128, KD * S], F8),
    ("wq8", [128, 6 * D], F8),       # th-major: [p, th, k, 128]
    ("wk8", [128, 6 * D], F8),
    ("wv8", [128, KD * D], F8),
    ("wp8", [128, KD * D], F8),
    ("gband", [128, 2 * 2 * BAND], BF),
    ("mrow", [128, MS], F32),
    ("w1_bf", [128, KD * Fd], BF),
    ("w2_bf", [128, KF * D], BF),
    ("bias_f", [128, KD + KF], F32),   # bq8 | b1, per-partition cols
    ("bias_b", [1, D], BF),            # b2 row
    ("gb", [128, 4 * D], BF),          # gamma1|beta1|gamma2|beta2 (host-bcast)
]

_BUILT = {}


def _build():
    if "nc" in _BUILT:
        return _BUILT["nc"]
    nc = bacc.Bacc("TRN2", target_bir_lowering=False, debug=False,
                   enable_asserts=False, num_devices=NCORES)
    io = {}
    for name, shape, dt in _SPECS:
        io[name] = nc.dram_tensor(name, shape, dt, kind="ExternalInput").ap()
    io["out"] = nc.dram_tensor("out", [S, D], F32, kind="ExternalOutput").ap()
    _trace(nc, io)
    nc.compile()
    _BUILT["nc"] = nc
    return nc


def _host_prep(inputs):
    bf = ml_dtypes.bfloat16
    f8 = ml_dtypes.float8_e4m3
    f32 = np.float32
    x = np.asarray(inputs["x"], f32)
    mask = np.asarray(inputs["mask"])

    def q8(a):
        return np.asarray(np.clip(a, -240.0, 240.0), f8)

    idx = np.arange(S, dtype=np.float64)
    dd = idx[None, :] - idx[:, None]
    sc = -0.5 * dd * dd
    sc -= sc.max(axis=-1, keepdims=True)
    e = np.exp(sc)
    gauss = e / e.sum(axis=-1, keepdims=True)  # [query s, key t], float64

    # band factor table: gband[t_loc, half, j, w] = exp(gauss[s, t]),
    # with t = 256*half + 128*j + t_loc, s = 256*half - 4 + w (1.0 if s OOB)
    t_loc = np.arange(128)
    gband = np.ones((128, 2, 2, BAND), np.float64)
    for half in range(2):
        for j in range(2):
            t_abs = 256 * half + 128 * j + t_loc          # [128]
            s_abs = 256 * half - 4 + np.arange(BAND)      # [BAND]
            valid = (s_abs >= 0) & (s_abs < S)
            gband[:, half, j, valid] = np.exp(gauss[s_abs[valid][None, :], t_abs[:, None]])
    gband = np.ascontiguousarray(gband.reshape(128, -1)).astype(bf)

    def sbl(a, p=128):  # [C*p, N] -> [p, C*N] (SBUF layout)
        cN = a.shape[0] // p
        return np.ascontiguousarray(
            a.reshape(cN, p, a.shape[1]).transpose(1, 0, 2).reshape(p, -1)
        )

    def thmaj(w):  # [D, D] -> [128, th, k, 128] flat (th-major fp8)
        a = np.asarray(w, f32).reshape(KD, 128, KD, 128)
        return np.ascontiguousarray(
            q8(a.transpose(1, 2, 0, 3)).reshape(128, -1)
        )

    def pcols(a, p=128):  # [C*p] -> [p, C] per-partition columns
        return np.ascontiguousarray(a.reshape(-1, p).T)

    bias_f = np.concatenate(
        [
            pcols(np.asarray(inputs["bq"], f32) * np.float32(0.125)),
            pcols(np.asarray(inputs["b1"], f32)),
        ],
        axis=1,
    )
    bias_b = np.asarray(inputs["b2"], f32).astype(bf)[None, :]
    # v/proj biases fold into the residual: x + h@wproj + bproj + bv@wproj
    x_fold = (
        np.asarray(inputs["b_proj"], f32)
        + np.asarray(inputs["bv"], f32) @ np.asarray(inputs["w_proj"], f32)
    )
    gb = np.ascontiguousarray(
        np.broadcast_to(
            np.concatenate(
                [
                    np.asarray(inputs["gamma1"], f32),
                    np.asarray(inputs["beta1"], f32),
                    np.asarray(inputs["gamma2"], f32),
                    np.asarray(inputs["beta2"], f32),
                ]
            )[None, :],
            (128, 4 * D),
        )
    ).astype(bf)
    shared = {
        "wq8": thmaj(inputs["wq"]),
        "wk8": thmaj(inputs["wk"]),
        "wv8": sbl(q8(np.asarray(inputs["wv"], f32))),
        "wp8": sbl(q8(np.asarray(inputs["w_proj"], f32))),
        "w1_bf": sbl(np.asarray(inputs["w1"], f32).astype(bf)),
        "w2_bf": sbl(np.asarray(inputs["w2"], f32).astype(bf)),
        "gband": gband,
        "bias_f": bias_f,
        "bias_b": bias_b,
        "gb": gb,
    }
    in_maps = []
    for b in range(NCORES):
        m = dict(shared)
        m["x"] = sbl(np.ascontiguousarray(x[b] + x_fold[None, :]).astype(bf))
        m["xT8"] = sbl(q8(np.ascontiguousarray(x[b].T)))
        m["mrow"] = np.ascontiguousarray(
            mask[b].astype(f32).reshape(MS, 128).T
        )
        in_maps.append(m)
    return in_maps


def _run(inputs, trace=False, trace_cores=None):
    nc = _build()
    in_maps = _host_prep(inputs)
    res = bass_utils.run_bass_kernel_spmd(
        nc, in_maps, core_ids=list(range(NCORES)), trace=trace,
        trace_cores=trace_cores,
    )
    out = np.stack([np.asarray(res.results[b]["out"]) for b in range(NCORES)])
    return out.astype(np.float32), res


def kernel(**inputs):
    return _run(inputs)[0]


# revision 64
# speedup vs baseline: 1.2183x; 1.0215x over previous
"""Trainium2 Bass kernel for a dense transformer block (B=8, S=512, D=768, H=12, Fd=3072).

Sharding: pure data-parallel over batch - one batch element per NeuronCore,
weights replicated, no collectives.

Key structure (v2):
  - Attention projections (q,k,v), the attn@v contraction and the output
    projection run in fp8e4m3 with DoubleRow perf mode (2 k-tiles per pass,
    2x PE throughput).  Scores (K=64) and the FFN stay bf16: fp8 would
    blow the 2e-2 error budget (measured 2.2e-2 for fp8 FFN in simulation),
    while qkv+attn+proj fp8 sims at 1.15e-2 total.
  - scoresT[t,s] layout (keys on partitions) so the key-padding mask is a
    per-partition factor: it folds into v (v rows and the sum-column get
    multiplied by mask), not into the softmax input.
  - The Gaussian positional bias enters as a multiplicative factor
    exp(gauss[s,t]) on exp(scores).  Since gauss < 2.3e-6 beyond |t-s| >= 4,
    the factor is 1.0 outside a 264-wide diagonal band: one small in-place
    band multiply per score tile instead of a full [512,512] multiply.
  - exp is computed with a fixed -3.5 shift (ACT bias) so values fit fp8
    range (max ~80 < 240); the shift cancels in the softmax normalization.
  - softmax sums come from a mask-valued extra column appended to v; the
    per-column reciprocal is broadcast across partitions with a 1-row
    f32r matmul (f32r: 1 cycle/row at N=512, no cast needed).
  - v/proj biases fold into the residual input on the host:
    x + b_proj + bv @ w_proj.  bk drops entirely (a per-query score shift
    cancels in softmax).  bq rides the qT activation bias, b1 the gelu
    bias, b2 a K=1 ones-row matmul.
  - Big weight DMAs are triggered from the Pool (gpsimd) queue (25ns per
    trigger vs 667ns on ACT/DVE), ordered so the first qk matmul only
    waits for xT + the first th-slice of wq.
"""

import numpy as np
import ml_dtypes

import concourse.bass as bass
import concourse.mybir as mybir
import concourse.tile as tile
from concourse.tile import add_dep_helper
from concourse import bacc
from concourse import bass_utils
from concourse.masks import make_identity

BF = mybir.dt.bfloat16
F32 = mybir.dt.float32
F32R = mybir.dt.float32r
F8 = mybir.dt.float8e4
AF = mybir.ActivationFunctionType
OP = mybir.AluOpType
DR = mybir.MatmulPerfMode.DoubleRow

B, S, D, H, Dh, Fd = 8, 512, 768, 12, 64, 3072
NCORES = 8
EPS = 1e-12
SHIFT = 3.5        # exp(score - SHIFT): keeps fp8 exp values < 240
BAND = 264         # gauss factor band width (|t-s|<=4 significant)

KD = D // 128      # 6  K-tiles over D
KP = KD // 2       # 3  K-tile pairs (DoubleRow)
MS = S // 128      # 4  chunks over sequence
KF = Fd // 128     # 24 K-tiles over Fd
NT = 2             # N-tiles over D for natural-layout outputs (2 x 384)
ND = D // NT       # 384


def _trace(nc, io):
    with tile.TileContext(nc) as tc:
        _trace_body(nc, tc, io)


def _act_recip(nc, out, in_):
    """ACT-engine reciprocal (InstActivation, func=Reciprocal).

    The bass wrapper refuses Reciprocal on ACT for accuracy reasons; the
    softmax normalizer only needs ~1e-2 relative accuracy, which the ACT
    table comfortably provides (validated against the reference output).
    """
    eng = nc.scalar
    inputs = [eng.lower_ap(in_)]
    for val in (0.0, 1.0, 0.0):  # bias, scale, alpha immediates
        inputs.append(mybir.ImmediateValue(dtype=mybir.dt.float32, value=val))
    return eng.add_instruction(
        mybir.InstActivation(
            name=eng.bass.get_next_instruction_name(),
            func=AF.Reciprocal,
            ins=inputs,
            outs=[eng.lower_ap(out)],
        )
    )


def _trace_body(nc, tc, io):
    from contextlib import ExitStack

    with ExitStack() as ctx:
        glob = ctx.enter_context(tc.tile_pool(name="glob", bufs=1))

        # ---- constants / small inputs (scalar queue: tiny, needed early) ----
        ident_f = glob.tile([128, 128], F32, tag="ident")
        make_identity(nc, ident_f)
        ones_bf = glob.tile([1, 512], BF, tag="ones_bf")
        nc.vector.memset(ones_bf, 1.0)
        ones64_b = glob.tile([1, 64], BF, tag="ones64")
        nc.vector.memset(ones64_b, 1.0)
        eps_t = glob.tile([128, 1], F32, tag="eps")
        nc.vector.memset(eps_t, EPS)
        nshift_t = glob.tile([128, 1], F32, tag="nshift")
        nc.vector.memset(nshift_t, -SHIFT)
        zero_t = glob.tile([128, 1], F32, tag="zero")
        nc.vector.memset(zero_t, 0.0)

        biasf_sb = glob.tile([128, KD + KF], F32, tag="biasf")
        nc.scalar.dma_start(out=biasf_sb, in_=io["bias_f"])
        bq8_c = biasf_sb[:, 0:KD]
        b1_c = biasf_sb[:, KD : KD + KF]

        biasb_sb = glob.tile([1, D], BF, tag="biasb")
        nc.scalar.dma_start(out=biasb_sb, in_=io["bias_b"])
        b2_r = biasb_sb[:, 0:D]

        mrow_sb = glob.tile([128, MS], F32, tag="mrow")
        nc.scalar.dma_start(out=mrow_sb, in_=io["mrow"])

        gbt = glob.tile([128, 4, D], BF, tag="gbt")
        g1b, be1b, g2b, be2b = gbt[:, 0, :], gbt[:, 1, :], gbt[:, 2, :], gbt[:, 3, :]

        w1_sb = glob.tile([128, KD, Fd], BF, tag="w1")
        h1_sb = glob.tile([128, MS, D], F32, tag="h1")
        h1T_sb = glob.tile([128, KD, S], BF, tag="h1T")
        x_sb = glob.tile([128, MS, D], BF, tag="x")

        # ================= attention scope =================
        with tc.tile_pool(name="attn", bufs=1) as attnp:
            xT_sb = attnp.tile([128, KD, S], F8, tag="xT")
            # th-slice 0 as separate tiles: tile-granular dependency tracking
            # means the first qk matmuls would otherwise wait for the whole
            # wq/wk transfer.
            wq0_sb = attnp.tile([128, 1, KD, 128], F8, tag="wq0")
            wk0_sb = attnp.tile([128, 1, KD, 128], F8, tag="wk0")
            wqr_sb = attnp.tile([128, 5, KD, 128], F8, tag="wqr")
            wkr_sb = attnp.tile([128, 5, KD, 128], F8, tag="wkr")
            wv_sb = attnp.tile([128, KD, D], F8, tag="wv")
            wp_sb = attnp.tile([128, KD, D], F8, tag="wp")
            gband_sb = attnp.tile([128, 2, 2, BAND], BF, tag="gband")

            def wq_sl(th):
                return (wq0_sb if th == 0 else wqr_sb)[:, 0 if th == 0 else th - 1]

            def wk_sl(th):
                return (wk0_sb if th == 0 else wkr_sb)[:, 0 if th == 0 else th - 1]

            # big-lead DMAs: descriptor gen costs ~0.65us per trigger, so
            # order by first use: xT+wq0+wk0 (first matmuls), wv (v tiles),
            # gband (first exp), then the bulk qk weights.
            nc.gpsimd.dma_start(
                out=xT_sb.rearrange("p c s -> p (c s)"), in_=io["xT8"]
            )
            nc.gpsimd.dma_start(
                out=wq0_sb.rearrange("p t c n -> p (t c n)"), in_=io["wq8"][:, 0:D]
            )
            nc.gpsimd.dma_start(
                out=wk0_sb.rearrange("p t c n -> p (t c n)"), in_=io["wk8"][:, 0:D]
            )
            nc.gpsimd.dma_start(out=wv_sb.rearrange("p c n -> p (c n)"), in_=io["wv8"])
            nc.gpsimd.dma_start(
                out=gband_sb.rearrange("p a b w -> p (a b w)"), in_=io["gband"]
            )
            nc.gpsimd.dma_start(
                out=wqr_sb.rearrange("p t c n -> p (t c n)"),
                in_=io["wq8"][:, D : 6 * D],
            )
            nc.gpsimd.dma_start(
                out=wkr_sb.rearrange("p t c n -> p (t c n)"),
                in_=io["wk8"][:, D : 6 * D],
            )
            late_dmas = [
                nc.sync.dma_start(out=wp_sb.rearrange("p c n -> p (c n)"), in_=io["wp8"]),
                nc.sync.dma_start(out=gbt.rearrange("p c n -> p (c n)"), in_=io["gb"]),
                nc.sync.dma_start(out=x_sb.rearrange("p c n -> p (c n)"), in_=io["x"]),
            ]

            qT_sb = attnp.tile([128, KD, S], BF, tag="qT")
            kT_sb = attnp.tile([128, KD, S], BF, tag="kT")
            # 128-wide per-head slots: v | mask-col x64. The 64 replicated
            # mask columns make the attn@v matmul emit the softmax sums
            # broadcast across partitions 64..127 for free (PE cost depends
            # only on N), so normalization needs no separate broadcast.
            v_sb = attnp.tile([128, MS, H, 128], F8, tag="v")
            nc.gpsimd.memset(v_sb[:, :, :, Dh:128], 1.0)
            hT_sb = attnp.tile([128, KD, S], F8, tag="hT")

            with tc.tile_pool(name="psA", bufs=1, space="PSUM") as psA:

                def qk_tile(th):
                    # bk dropped: a per-query score shift cancels in softmax.
                    act = None
                    for w_sl, dst in ((wq_sl(th), qT_sb), (wk_sl(th), kT_sb)):
                        ps = psA.tile([128, 512], F32, tag="acc", bufs=2, name="ps_qk")
                        for j in range(KP):
                            nc.tensor.matmul(
                                ps, w_sl[:, 2 * j : 2 * j + 2, :],
                                xT_sb[:, 2 * j : 2 * j + 2, :],
                                start=(j == 0), stop=(j == KP - 1), perf_mode=DR,
                            )
                        if dst is qT_sb:
                            act = nc.scalar.activation(
                                out=dst[:, th, :], in_=ps, func=AF.Identity,
                                bias=bq8_c[:, th : th + 1], scale=0.125,
                            )
                        else:
                            act = nc.vector.tensor_copy(out=dst[:, th, :], in_=ps)
                    return act

                def v_tiles(n):
                    # bv host-folded into x; mask folded into v rows (and the
                    # sum column) so the softmax input needs no mask at all.
                    for c in range(MS):
                        ps = psA.tile([128, ND], F32, tag="acc", bufs=2, name="ps_v")
                        for j in range(KP):
                            nc.tensor.matmul(
                                ps, xT_sb[:, 2 * j : 2 * j + 2, 128 * c : 128 * (c + 1)],
                                wv_sb[:, 2 * j : 2 * j + 2, ND * n : ND * (n + 1)],
                                start=(j == 0), stop=(j == KP - 1), perf_mode=DR,
                            )
                        nc.vector.tensor_scalar(
                            out=v_sb[:, c, 6 * n : 6 * (n + 1), 0:Dh],
                            in0=ps.rearrange("p (h d) -> p h d", d=Dh),
                            scalar1=mrow_sb[:, c : c + 1], scalar2=None,
                            op0=OP.mult,
                        )
                    if n == 1:
                        for c in range(MS):
                            nc.vector.tensor_scalar(
                                out=v_sb[:, c, :, Dh:128],
                                in0=v_sb[:, c, :, Dh:128],
                                scalar1=mrow_sb[:, c : c + 1],
                                scalar2=None, op0=OP.mult,
                            )

                def scores_exp(h):
                    th, off = h // 2, (h % 2) * 64
                    qh = qT_sb[off : off + 64, th, :]
                    kh = kT_sb[off : off + 64, th, :]
                    exp_tiles = []
                    for half in range(2):
                        ex = attnp.tile([128, 2, 512], F8, tag="exp", bufs=8, name="ex")
                        for j in range(2):
                            c = 2 * half + j
                            ps_sc = psA.tile([128, 512], F32, tag="sc2", bufs=4, name="ps_sc")
                            nc.tensor.matmul(
                                ps_sc, kh[:, 128 * c : 128 * (c + 1)], qh,
                                start=True, stop=True,
                            )
                            nc.scalar.activation(
                                out=ex[:, j, :], in_=ps_sc, func=AF.Exp, bias=nshift_t
                            )
                        # gauss factor: identity outside a 264-wide diagonal band
                        lo = 256 * half - 4
                        wlo, whi = max(0, -lo), min(BAND, 512 - lo)
                        eng = nc.gpsimd if half == 1 else nc.vector
                        eng.tensor_tensor(
                            out=ex[:, :, lo + wlo : lo + whi],
                            in0=ex[:, :, lo + wlo : lo + whi],
                            in1=gband_sb[:, half, :, wlo:whi], op=OP.mult,
                        )
                        exp_tiles.append(ex)
                    return exp_tiles

                def h_chain(h, exp_tiles):
                    ps_h = psA.tile([128, 512], F32, tag="hT", bufs=2, name="ps_h")
                    for half in range(2):
                        nc.tensor.matmul(
                            ps_h, v_sb[:, 2 * half : 2 * half + 2, h, :],
                            exp_tiles[half],
                            start=(half == 0), stop=(half == 1), perf_mode=DR,
                        )
                    # sums (rows 64..127, already partition-broadcast by the
                    # matmul) to SBUF, then a [64,512] reciprocal: approx
                    # cost is partition-blind, so this is as cheap as [1,512].
                    sums = attnp.tile([64, 512], F32, tag="sums", bufs=2, name="sums")
                    nc.scalar.copy(sums, ps_h[Dh:128, :])
                    rec = attnp.tile([64, 512], F32, tag="rec", bufs=2, name="rec")
                    nc.vector.reciprocal_approx_fast(out=rec, in_=sums)
                    return ps_h, rec

                def h_apply(h, ps_h, rec):
                    th, off = h // 2, (h % 2) * 64
                    return nc.vector.tensor_tensor(
                        out=hT_sb[off : off + 64, th, :], in0=ps_h[0:Dh, :],
                        in1=rec, op=OP.mult,
                    )

                # software-pipelined head loop: per iteration each engine
                # sees at most one stage of each kind, so stage latencies of
                # consecutive heads overlap instead of serializing.
                qk_acts = [qk_tile(0), qk_tile(1)]
                # not needed until proj/LN1: keep them off the DMA rings
                # while the attention lead-in streams in
                for dma in late_dmas:
                    add_dep_helper(dma.ins, qk_acts[0].ins, True, "defer until lead-in clear")
                exps, chains = {}, {}
                exps[0] = scores_exp(0)
                v_tiles(0)
                exps[1] = scores_exp(1)
                v_tiles(1)
                last_mult = None
                for h in range(H + 2):
                    if h < H and h not in exps:
                        exps[h] = scores_exp(h)
                        if 2 <= h <= 5:
                            qk_acts.append(qk_tile(h))
                    if 1 <= h <= H:
                        chains[h - 1] = h_chain(h - 1, exps.pop(h - 1))
                    if h >= 2:
                        last_mult = h_apply(h - 2, *chains.pop(h - 2))

                # defer the big FFN weight DMAs until the lead-in is clear;
                # rings are idle mid-attention, so anchor on the 3rd qk tile
                w1dma = nc.sync.dma_start(out=w1_sb.rearrange("p c n -> p (c n)"), in_=io["w1_bf"])
                add_dep_helper(w1dma.ins, qk_acts[2].ins, True, "defer w1 until lead-in clear")

                # --- proj (fp8 DoubleRow) + residual + LN1 + h1 transpose,
                # software-pipelined: transposes of chunk m-1 fill the PE
                # while chunk m's LN chain runs on DVE/ACT ---
                def proj_chunk(m):
                    pss = []
                    for n in range(NT):
                        ps = psA.tile([128, ND], F32, tag="acc", bufs=2, name="ps_pr")
                        for j in range(KP):
                            nc.tensor.matmul(
                                ps, hT_sb[:, 2 * j : 2 * j + 2, 128 * m : 128 * (m + 1)],
                                wp_sb[:, 2 * j : 2 * j + 2, ND * n : ND * (n + 1)],
                                start=(j == 0), stop=(j == KP - 1), perf_mode=DR,
                            )
                        pss.append(ps)
                    row = glob.tile([128, D], F32, tag="rowtmp", bufs=2, name="row")
                    for n in range(NT):
                        nc.vector.tensor_tensor(
                            out=row[:, ND * n : ND * (n + 1)], in0=pss[n],
                            in1=x_sb[:, m, ND * n : ND * (n + 1)], op=OP.add,
                        )
                    _layernorm(nc, glob, row, g1b, be1b, eps_t, h1_sb[:, m, :])

                def transpose_chunk(m):
                    for f in range(KD):
                        ps_t = psA.tile([128, 128], F32, tag="acc", bufs=2, name="ps_t")
                        nc.tensor.transpose(ps_t, h1_sb[:, m, 128 * f : 128 * (f + 1)], ident_f)
                        nc.scalar.copy(out=h1T_sb[:, f, 128 * m : 128 * (m + 1)], in_=ps_t)

                proj_chunk(0)
                proj_chunk(1)
                proj_chunk(2)
                transpose_chunk(0)
                proj_chunk(3)
                transpose_chunk(1)
                transpose_chunk(2)
                transpose_chunk(3)

        # ================= FFN scope (bf16) =================
        with tc.tile_pool(name="ffn", bufs=1) as ffnp, \
             tc.tile_pool(name="psF", bufs=1, space="PSUM") as psF:
            ff1T_sb = ffnp.tile([128, KF, S], BF, tag="ff1T")
            w2_sb = ffnp.tile([128, KF, D], BF, tag="w2")
            w2dma = nc.sync.dma_start(out=w2_sb.rearrange("p c n -> p (c n)"), in_=io["w2_bf"])
            add_dep_helper(w2dma.ins, last_mult.ins, True, "defer w2 until attention done")
            # two half-S passes: the first needs only h1 chunks 0-1, so it
            # overlaps the LN1/transpose ramp of chunks 2-3.
            for half in range(2):
                sl = slice(256 * half, 256 * (half + 1))
                for fm in range(KF):
                    ps = psF.tile([128, 256], F32, tag="acc", bufs=8, name="ps_f1")
                    for k in range(KD):
                        nc.tensor.matmul(
                            ps, w1_sb[:, k, 128 * fm : 128 * (fm + 1)],
                            h1T_sb[:, k, sl],
                            start=(k == 0), stop=(k == KD - 1),
                        )
                    nc.scalar.activation(
                        out=ff1T_sb[:, fm, sl], in_=ps, func=AF.Gelu,
                        bias=b1_c[:, fm : fm + 1], scale=1.0,
                    )

            for m in range(MS):
                pss = []
                for n in range(NT):
                    ps = psF.tile([128, ND], F32, tag="acc", bufs=8, name="ps_f2")
                    for k in range(KF):
                        nc.tensor.matmul(
                            ps, ff1T_sb[:, k, 128 * m : 128 * (m + 1)],
                            w2_sb[:, k, ND * n : ND * (n + 1)],
                            start=(k == 0), stop=False,
                        )
                    nc.tensor.matmul(
                        ps, ones_bf[:, 0:128], b2_r[:, ND * n : ND * (n + 1)],
                        start=False, stop=True,
                    )
                    pss.append(ps)
                row = glob.tile([128, D], F32, tag="rowtmp", bufs=2, name="row2")
                for n in range(NT):
                    nc.vector.tensor_tensor(
                        out=row[:, ND * n : ND * (n + 1)], in0=pss[n],
                        in1=h1_sb[:, m, ND * n : ND * (n + 1)], op=OP.add,
                    )
                outrow = glob.tile([128, D], F32, tag="outrow", bufs=2, name="outrow")
                _layernorm(nc, glob, row, g2b, be2b, eps_t, outrow)
                nc.gpsimd.dma_start(
                    out=io["out"][128 * m : 128 * (m + 1), :], in_=outrow
                )


def _layernorm(nc, pool, row, gamma_b, beta_b, eps_t, out_ap):
    st = pool.tile([128, 3, 6], F32, tag="st", bufs=2, name="st")
    for g in range(3):
        nc.vector.bn_stats(out=st[:, g, :], in_=row[:, 256 * g : 256 * (g + 1)])
    mv = pool.tile([128, 2], F32, tag="mv", bufs=2, name="mv")
    nc.vector.bn_aggr(out=mv, in_=st)
    sd = pool.tile([128, 1], F32, tag="sd", bufs=2, name="sd")
    nc.scalar.activation(out=sd, in_=mv[:, 1:2], func=AF.Sqrt, bias=eps_t, scale=1.0)
    rs = pool.tile([128, 1], F32, tag="rs", bufs=2, name="rs")
    nc.vector.reciprocal(rs, sd)
    # in-place: row = (row - mean) * gamma ; out = row * rstd + beta
    nc.vector.scalar_tensor_tensor(
        out=row, in0=row, scalar=mv[:, 0:1], in1=gamma_b, op0=OP.subtract, op1=OP.mult
    )
    nc.vector.scalar_tensor_tensor(
        out=out_ap, in0=row, scalar=rs, in1=beta_b, op0=OP.mult, op1=OP.add
    )


_SPECS = [
    # (name, shape, dtype) - big tensors pre-permuted on host to SBUF layout
    ("x", [128, MS * D], BF),
    ("xT8", [128, KD * S], F8),
    ("wq8", [128, 6 * D], F8),       # th-major: [p, th, k, 128]
    ("wk8", [128, 6 * D], F8),
    ("wv8", [128, KD * D], F8),
    ("wp8", [128, KD * D], F8),
    ("gband", [128, 2 * 2 * BAND], BF),
    ("mrow", [128, MS], F32),
    ("w1_bf", [128, KD * Fd], BF),
    ("w2_bf", [128, KF * D], BF),
    ("bias_f", [128, KD + KF], F32),   # bq8 | b1, per-partition cols
    ("bias_b", [1, D], BF),            # b2 row
    ("gb", [128, 4 * D], BF),          # gamma1|beta1|gamma2|beta2 (host-bcast)
]

_BUILT = {}


def _build():
    if "nc" in _BUILT:
        return _BUILT["nc"]
    nc = bacc.Bacc("TRN2", target_bir_lowering=False, debug=False,
                   enable_asserts=False, num_devices=NCORES)
    io = {}
    for name, shape, dt in _SPECS:
        io[name] = nc.dram_tensor(name, shape, dt, kind="ExternalInput").ap()
    io["out"] = nc.dram_tensor("out", [S, D], F32, kind="ExternalOutput").ap()
    _trace(nc, io)
    nc.compile()
    _BUILT["nc"] = nc
    return nc


def _host_prep(inputs):
    bf = ml_dtypes.bfloat16
    f8 = ml_dtypes.float8_e4m3
    f32 = np.float32
    x = np.asarray(inputs["x"], f32)
    mask = np.asarray(inputs["mask"])

    def q8(a):
        return np.asarray(np.clip(a, -240.0, 240.0), f8)

    idx = np.arange(S, dtype=np.float64)
    dd = idx[None, :] - idx[:, None]
    sc = -0.5 * dd * dd
    sc -= sc.max(axis=-1, keepdims=True)
    e = np.exp(sc)
    gauss = e / e.sum(axis=-1, keepdims=True)  # [query s, key t], float64

    # band factor table: gband[t_loc, half, j, w] = exp(gauss[s, t]),
    # with t = 256*half + 128*j + t_loc, s = 256*half - 4 + w (1.0 if s OOB)
    t_loc = np.arange(128)
    gband = np.ones((128, 2, 2, BAND), np.float64)
    for half in range(2):
        for j in range(2):
            t_abs = 256 * half + 128 * j + t_loc          # [128]
            s_abs = 256 * half - 4 + np.arange(BAND)      # [BAND]
            valid = (s_abs >= 0) & (s_abs < S)
            gband[:, half, j, valid] = np.exp(gauss[s_abs[valid][None, :], t_abs[:, None]])
    gband = np.ascontiguousarray(gband.reshape(128, -1)).astype(bf)

    def sbl(a, p=128):  # [C*p, N] -> [p, C*N] (SBUF layout)
        cN = a.shape[0] // p
        return np.ascontiguousarray(
            a.reshape(cN, p, a.shape[1]).transpose(1, 0, 2).reshape(p, -1)
        )

    def thmaj(w):  # [D, D] -> [128, th, k, 128] flat (th-major fp8)
        a = np.asarray(w, f32).reshape(KD, 128, KD, 128)
        return np.ascontiguousarray(
            q8(a.transpose(1, 2, 0, 3)).reshape(128, -1)
        )

    def pcols(a, p=128):  # [C*p] -> [p, C] per-partition columns
        return np.ascontiguousarray(a.reshape(-1, p).T)

    bias_f = np.concatenate(
        [
            pcols(np.asarray(inputs["bq"], f32) * np.float32(0.125)),
            pcols(np.asarray(inputs["b1"], f32)),
        ],
        axis=1,
    )
    bias_b = np.asarray(inputs["b2"], f32).astype(bf)[None, :]
    # v/proj biases fold into the residual: x + h@wproj + bproj + bv@wproj
    x_fold = (
        np.asarray(inputs["b_proj"], f32)
        + np.asarray(inputs["bv"], f32) @ np.asarray(inputs["w_proj"], f32)
    )
    gb = np.ascontiguousarray(
        np.broadcast_to(
            np.concatenate(
                [
                    np.asarray(inputs["gamma1"], f32),
                    np.asarray(inputs["beta1"], f32),
                    np.asarray(inputs["gamma2"], f32),
                    np.asarray(inputs["beta2"], f32),
                ]
            )[None, :],
            (128, 4 * D),
        )
    ).astype(bf)
    shared = {
        "wq8": thmaj(inputs["wq"]),
        "wk8": thmaj(inputs["wk"]),
        "wv8": sbl(q8(np.asarray(inputs["wv"], f32))),
        "wp8": sbl(q8(np.asarray(inputs["w_proj"], f32))),
        "w1_bf": sbl(np.asarray(inputs["w1"], f32).astype(bf)),
        "w2_bf": sbl(np.asarray(inputs["w2"], f32).astype(bf)),
        "gband": gband,
        "bias_f": bias_f,
        "bias_b": bias_b,
        "gb": gb,
    }
    in_maps = []
    for b in range(NCORES):
        m = dict(shared)
        m["x"] = sbl(np.ascontiguousarray(x[b] + x_fold[None, :]).astype(bf))
        m["xT8"] = sbl(q8(np.ascontiguousarray(x[b].T)))
        m["mrow"] = np.ascontiguousarray(
            mask[b].astype(f32).reshape(MS, 128).T
        )
        in_maps.append(m)
    return in_maps


def _run(inputs, trace=False, trace_cores=None):
    nc = _build()
    in_maps = _host_prep(inputs)
    res = bass_utils.run_bass_kernel_spmd(
        nc, in_maps, core_ids=list(range(NCORES)), trace=trace,
        trace_cores=trace_cores,
    )
    out = np.stack([np.asarray(res.results[b]["out"]) for b in range(NCORES)])
    return out.astype(np.float32), res


def kernel(**inputs):
    return _run(inputs)[0]
